# revision 1
# baseline (speedup 1.0000x reference)
"""MQA attention (LN + QKV proj + RoPE + causal attn + out-proj) on 8 trn2 cores.

Sharding: tensor-parallel over heads (2 heads/core, Wq cols + Wo rows), KV
replicated (single KV head), out-proj produces per-core partial sums that the
host reduces.

Per-core dataflow (all tokens, bf16 matmuls, f32 accumulation):
  LN(x) natural layout -> PE-transpose to xn^T -> q^T/k^T/v^T projections
  -> RoPE on q^T,k^T -> S^T = k @ q^T per (batch,head) causal-blocked
  -> exp on ScalarE (scale folded in, no max subtraction: |S*scale| <= ~15)
  -> AV: out^T = V^T E^T accumulated over k-tiles; row sums via ones-matmul
  -> normalize -> out-proj partial [dim rows of this core's heads] -> host sum.
"""

import sys

if "/opt/trn_rl_repo" not in sys.path:
    sys.path.insert(0, "/opt/trn_rl_repo")

import ml_dtypes
import numpy as np

import concourse.bass as bass
import concourse.tile as tile
from concourse import bacc, mybir
from concourse.masks import make_identity

F32 = mybir.dt.float32
DT = mybir.dt.bfloat16  # matmul operand storage dtype
DT_NP = ml_dtypes.bfloat16

B, N, DIM, DH, HEADS = 2, 2048, 2048, 128, 16
H_LOCAL = 2  # heads per core
N_CORES = 8
KT = DIM // 128  # k-tiles over the model dim
TT = N // 128  # token tiles per batch
CHUNK = 512  # token chunk for projection phase
NCH = N // CHUNK  # chunks per batch
QG = 512  # q-group width in attention
NQG = N // QG
SCALE = float(DH) ** -0.5
EPS = 1e-5
NEG = -1e30


def build_nc(repeat=1, phase=4):
    nc = bacc.Bacc(None, target_bir_lowering=False, debug=False)

    x_d = nc.dram_tensor("x_in", [B, N, DIM], DT, kind="ExternalInput")
    wq_d = nc.dram_tensor("wq", [128, KT, H_LOCAL * DH], DT, kind="ExternalInput")
    wk_d = nc.dram_tensor("wk", [128, KT, DH], DT, kind="ExternalInput")
    wv_d = nc.dram_tensor("wv", [128, KT, DH], DT, kind="ExternalInput")
    wo_d = nc.dram_tensor("wo", [128, H_LOCAL, DIM], DT, kind="ExternalInput")
    cos_d = nc.dram_tensor("cosT", [DH, N], F32, kind="ExternalInput")
    sin_d = nc.dram_tensor("sinT", [DH, N], F32, kind="ExternalInput")
    msk_d = nc.dram_tensor("mask", [128, 128], F32, kind="ExternalInput")
    scr_d = nc.dram_tensor("scratch", [B, NCH, 4, 128, 2], F32, kind="ExternalOutput")
    out_d = nc.dram_tensor("out_partial", [B, N, DIM], DT, kind="ExternalOutput")

    with tile.TileContext(nc) as tc:
        with (
            tc.tile_pool(name="const", bufs=1) as const,
            tc.tile_pool(name="xp", bufs=2) as xp,
            tc.tile_pool(name="xnp", bufs=3) as xnp,
            tc.tile_pool(name="xtp", bufs=2) as xtp,
            tc.tile_pool(name="store", bufs=1) as store,
            tc.tile_pool(name="small", bufs=4) as small,
            tc.tile_pool(name="rope", bufs=4) as ropep,
            tc.tile_pool(name="ep", bufs=3) as ep,
            tc.tile_pool(name="bounce", bufs=2) as bounce,
            tc.tile_pool(name="op", bufs=3) as op,
            tc.tile_pool(name="ps", bufs=1, space="PSUM") as ps,
        ):
            # --- constants ---
            wq_sb = const.tile([128, KT, H_LOCAL * DH], DT)
            nc.sync.dma_start(wq_sb[:], wq_d[:])
            wk_sb = const.tile([128, KT, DH], DT)
            nc.sync.dma_start(wk_sb[:], wk_d[:])
            wv_sb = const.tile([128, KT, DH], DT)
            nc.sync.dma_start(wv_sb[:], wv_d[:])
            wo_sb = const.tile([128, H_LOCAL, DIM], DT)
            nc.sync.dma_start(wo_sb[:], wo_d[:])
            cos_sb = const.tile([DH, N], F32)
            nc.sync.dma_start(cos_sb[:], cos_d[:])
            sin_sb = const.tile([DH, N], F32)
            nc.sync.dma_start(sin_sb[:], sin_d[:])
            msk_sb = const.tile([128, 128], F32)
            nc.sync.dma_start(msk_sb[:], msk_d[:])
            ident = const.tile([128, 128], DT)
            make_identity(nc, ident)
            ones_mm = const.tile([128, 1], DT)
            nc.vector.memset(ones_mm, 1.0)
            eps_t = const.tile([128, 1], F32)
            nc.vector.memset(eps_t, EPS)

            # --- persistent activations ---
            qT_sb = store.tile([DH, H_LOCAL, B, N], DT, tag="qT")
            kT_sb = store.tile([DH, B, N], DT, tag="kT")
            v_sb = store.tile([128, B, TT, DH], DT, tag="v")
            aoT_sb = store.tile([DH, H_LOCAL, B, N], DT, tag="aoT")

            def rope_evict(dst, src_ps, t0, t1):
                # dst = src*cos + rotate_half(src)*sin_signed, src is [128, n] PSUM
                n = t1 - t0
                rot = ropep.tile([DH, CHUNK], F32, tag="rot")
                nc.scalar.copy(rot[0:64, :n], src_ps[64:128, :])
                nc.scalar.copy(rot[64:128, :n], src_ps[0:64, :])
                tmp = ropep.tile([DH, CHUNK], F32, tag="tmp")
                nc.vector.tensor_mul(tmp[:, :n], src_ps[:], cos_sb[:, t0:t1])
                nc.vector.tensor_mul(rot[:, :n], rot[:, :n], sin_sb[:, t0:t1])
                nc.vector.tensor_add(dst, tmp[:, :n], rot[:, :n])

            for _rep, b in [(r, bb) for r in range(repeat) for bb in range(B)]:
                # ---- LN + transpose + projections + RoPE, per 512-token chunk ----
                for cg in range(NCH):
                    c0 = cg * CHUNK
                    xnT = xtp.tile([128, KT, CHUNK], DT, tag="xnT")
                    xts = []
                    mr = small.tile([128, 4, 2], F32, tag="mr")
                    for t in range(CHUNK // 128):
                        tok0 = c0 + t * 128
                        x_t = xp.tile([128, DIM], DT, tag="x", bufs=5)
                        xts.append(x_t)
                        nc.sync.dma_start(x_t[:], x_d[b, tok0 : tok0 + 128, :])
                        stats = small.tile([128, 4, 6], F32, tag="stats")
                        for i in range(4):
                            nc.vector.bn_stats(
                                out=stats[:, i, :], in_=x_t[:, i * 512 : (i + 1) * 512]
                            )
                        nc.vector.bn_aggr(out=mr[:, t, :], in_=stats[:])
                    # one batched sqrt per chunk (minimizes ACT table reloads)
                    rstd4 = small.tile([128, 4], F32, tag="rstd4")
                    nc.scalar.activation(
                        out=rstd4[:],
                        in_=mr[:, :, 1],
                        func=mybir.ActivationFunctionType.Sqrt,
                        bias=eps_t[:],
                    )
                    nc.vector.reciprocal(out=rstd4[:], in_=rstd4[:])
                    if phase < 1:
                        nc.sync.dma_start(scr_d[b, cg, 0], mr[:, 0, :])
                        nc.sync.dma_start(scr_d[b, cg, 1], rstd4[:, 0:2])
                        continue
                    for t in range(CHUNK // 128):
                        tok0 = c0 + t * 128
                        x_t = xts[t]
                        mv = None
                        xn_t = xnp.tile([128, DIM], DT, tag="xn")
                        if t % 2 == 0:
                            nc.vector.tensor_scalar(
                                out=xn_t[:],
                                in0=x_t[:],
                                scalar1=mr[:, t, 0:1],
                                scalar2=rstd4[:, t : t + 1],
                                op0=mybir.AluOpType.subtract,
                                op1=mybir.AluOpType.mult,
                            )
                        else:
                            negmur = small.tile([128, 1], F32, tag="negmur")
                            nc.vector.tensor_scalar(
                                out=negmur[:],
                                in0=mr[:, t, 0:1],
                                scalar1=rstd4[:, t : t + 1],
                                scalar2=-1.0,
                                op0=mybir.AluOpType.mult,
                                op1=mybir.AluOpType.mult,
                            )
                            nc.scalar.activation(
                                out=xn_t[:],
                                in_=x_t[:],
                                func=mybir.ActivationFunctionType.Identity,
                                bias=negmur[:],
                                scale=rstd4[:, t : t + 1],
                            )
                        if t % 2 == 0:
                            # PE transpose via regular matmul with identity moving
                            for g in range(4):
                                tp_ps = ps.tile([128, 512], F32, tag="s", bufs=2)
                                for j in range(4):
                                    kt = g * 4 + j
                                    nc.tensor.matmul(
                                        tp_ps[:, j * 128 : (j + 1) * 128],
                                        xn_t[:, kt * 128 : (kt + 1) * 128],
                                        ident[:],
                                    )
                                dst = xnT[:, g * 4 : (g + 1) * 4, t * 128 : (t + 1) * 128]
                                src = tp_ps[:].rearrange("p (k t) -> p k t", k=4)
                                if g % 2 == 0:
                                    nc.scalar.copy(dst, src)
                                else:
                                    nc.vector.tensor_copy(dst, src)
                        else:
                            # DMA xbar transpose (parallel resource)
                            nc.scalar.dma_start_transpose(
                                xnT[:, :, t * 128 : (t + 1) * 128], xn_t[:]
                            )

                    if phase < 2:
                        continue
                    # projections: q^T (2 heads), k^T, v^T over this chunk
                    qt0 = ps.tile([DH, CHUNK], F32, tag="acc", bufs=4)
                    qt1 = ps.tile([DH, CHUNK], F32, tag="acc", bufs=4)
                    ktp = ps.tile([DH, CHUNK], F32, tag="acc", bufs=4)
                    vtp = ps.tile([DH, CHUNK], F32, tag="acc", bufs=4)
                    for kt in range(KT):
                        rhs = xnT[:, kt, :]
                        nc.tensor.matmul(
                            qt0[:], wq_sb[:, kt, 0:128], rhs,
                            start=(kt == 0), stop=(kt == KT - 1),
                        )
                        nc.tensor.matmul(
                            qt1[:], wq_sb[:, kt, 128:256], rhs,
                            start=(kt == 0), stop=(kt == KT - 1),
                        )
                        nc.tensor.matmul(
                            ktp[:], wk_sb[:, kt, :], rhs,
                            start=(kt == 0), stop=(kt == KT - 1),
                        )
                        nc.tensor.matmul(
                            vtp[:], wv_sb[:, kt, :], rhs,
                            start=(kt == 0), stop=(kt == KT - 1),
                        )
                    rope_evict(qT_sb[:, 0, b, c0 : c0 + CHUNK], qt0, c0, c0 + CHUNK)
                    rope_evict(qT_sb[:, 1, b, c0 : c0 + CHUNK], qt1, c0, c0 + CHUNK)
                    rope_evict(kT_sb[:, b, c0 : c0 + CHUNK], ktp, c0, c0 + CHUNK)
                    # v: evict v^T then PE-transpose to natural [tok, dh] tiles
                    vT_sb = bounce.tile([DH, CHUNK], DT, tag="vT")
                    nc.scalar.copy(vT_sb[:], vtp[:])
                    vn_ps = ps.tile([128, 512], F32, tag="s", bufs=2)
                    for tv in range(4):
                        nc.tensor.matmul(
                            vn_ps[:, tv * 128 : (tv + 1) * 128],
                            vT_sb[:, tv * 128 : (tv + 1) * 128],
                            ident[:],
                        )
                    nc.scalar.copy(
                        v_sb[:, b, cg * 4 : (cg + 1) * 4, :],
                        vn_ps[:].rearrange("p (t d) -> p t d", t=4),
                    )

                # ---- attention for batch b ----
                if phase < 3:
                    continue
                for h in range(H_LOCAL):
                    for qg in range(NQG):
                        q0 = qg * QG
                        nkt = (qg + 1) * (QG // 128)
                        avT = ps.tile([DH, QG], F32, tag="av", bufs=1)
                        sums = ps.tile([1, QG], F32, tag="sums", bufs=1)
                        for kt in range(nkt):
                            off = max(0, kt * 128 - q0)
                            st = ps.tile([128, QG], F32, tag="s", bufs=2)
                            nc.tensor.matmul(
                                st[:, off:],
                                kT_sb[:, b, kt * 128 : (kt + 1) * 128],
                                qT_sb[:, h, b, q0 + off : q0 + QG],
                            )
                            if kt * 128 >= q0:  # diagonal block: causal mask
                                nc.vector.tensor_add(
                                    st[:, off : off + 128],
                                    st[:, off : off + 128],
                                    msk_sb[:],
                                )
                            et = ep.tile([128, QG], DT, tag="et")
                            nc.scalar.activation(
                                out=et[:, off:],
                                in_=st[:, off:],
                                func=mybir.ActivationFunctionType.Exp,
                                scale=SCALE,
                            )
                            nc.tensor.matmul(
                                avT[:, off:],
                                v_sb[:, b, kt, :],
                                et[:, off:],
                                start=(kt == 0),
                                stop=(kt == nkt - 1),
                            )
                            nc.tensor.matmul(
                                sums[:, off:],
                                ones_mm[:],
                                et[:, off:],
                                start=(kt == 0),
                                stop=(kt == nkt - 1),
                            )
                        recip = small.tile([1, QG], F32, tag="recip")
                        nc.vector.reciprocal(out=recip[:], in_=sums[:])
                        rbc = bounce.tile([128, QG], F32, tag="rbc")
                        nc.gpsimd.partition_broadcast(rbc[:], recip[:])
                        nc.vector.tensor_mul(
                            aoT_sb[:, h, b, q0 : q0 + QG], avT[:], rbc[:]
                        )

                # ---- out-proj partial for batch b ----
                if phase < 4:
                    continue
                for tt in range(TT):
                    for dg in range(4):
                        opp = ps.tile([128, 512], F32, tag="acc", bufs=4)
                        for h in range(H_LOCAL):
                            nc.tensor.matmul(
                                opp[:],
                                aoT_sb[:, h, b, tt * 128 : (tt + 1) * 128],
                                wo_sb[:, h, dg * 512 : (dg + 1) * 512],
                                start=(h == 0),
                                stop=(h == H_LOCAL - 1),
                            )
                        ot = op.tile([128, 512], DT, tag="ot")
                        if dg % 2 == 0:
                            nc.scalar.copy(ot[:], opp[:])
                        else:
                            nc.vector.tensor_copy(ot[:], opp[:])
                        nc.sync.dma_start(
                            out_d[b, tt * 128 : (tt + 1) * 128, dg * 512 : (dg + 1) * 512],
                            ot[:],
                        )

    nc.compile()
    return nc


def make_in_maps(x, gamma, Wq, Wkv, Wo):
    x = np.ascontiguousarray(np.asarray(x, dtype=np.float32).astype(DT_NP))
    g = np.asarray(gamma, dtype=np.float32)
    Wq = np.asarray(Wq, dtype=np.float32) * g[:, None]
    Wkv = np.asarray(Wkv, dtype=np.float32) * g[:, None]
    Wo = np.asarray(Wo, dtype=np.float32)

    t = np.arange(N, dtype=np.float64)
    inv = 1.0 / (10000.0 ** (np.arange(0, DH, 2, dtype=np.float64) / DH))  # [64]
    fr = np.outer(inv, t)  # [d, t]
    cosT = np.concatenate([np.cos(fr), np.cos(fr)], 0).astype(np.float32)
    sinT = np.concatenate([-np.sin(fr), np.sin(fr)], 0).astype(np.float32)
    mask = np.where(
        np.arange(128)[:, None] > np.arange(128)[None, :], NEG, 0.0
    ).astype(np.float32)

    def pt(w):  # [DIM, M] -> [128, KT, M] partition-major
        return np.ascontiguousarray(
            w.reshape(KT, 128, -1).transpose(1, 0, 2).astype(DT_NP)
        )

    Wk = Wkv[:, :DH]
    Wv = Wkv[:, DH:]
    maps = []
    for c in range(N_CORES):
        wq_c = pt(Wq[:, c * H_LOCAL * DH : (c + 1) * H_LOCAL * DH])
        wo_c = np.ascontiguousarray(
            Wo[c * H_LOCAL * DH : (c + 1) * H_LOCAL * DH]
            .reshape(H_LOCAL, DH, DIM)
            .transpose(1, 0, 2)
            .astype(DT_NP)
        )
        maps.append(
            {
                "x_in": x,
                "wq": wq_c,
                "wk": pt(Wk),
                "wv": pt(Wv),
                "wo": wo_c,
                "cosT": cosT,
                "sinT": sinT,
                "mask": mask,
            }
        )
    return maps


_NC_CACHE = {}


def get_nc(repeat=1, phase=4):
    key = (repeat, phase)
    if key not in _NC_CACHE:
        _NC_CACHE[key] = build_nc(repeat, phase)
    return _NC_CACHE[key]


def kernel(x, gamma, Wq, Wkv, Wo, _trace=False, _repeat=1):
    from concourse import bass_utils

    nc = get_nc(_repeat)
    in_maps = make_in_maps(x, gamma, Wq, Wkv, Wo)
    res = bass_utils.run_bass_kernel_spmd(
        nc, in_maps, core_ids=list(range(N_CORES)), trace=_trace
    )
    out = np.zeros((B, N, DIM), dtype=np.float32)
    for r in res.results:
        out += np.asarray(r["out_partial"], dtype=np.float32)
    if _trace:
        kernel.last_results = res
    return out



# revision 6
# speedup vs baseline: 1.2894x; 1.2894x over previous
"""MQA attention (LN + QKV proj + RoPE + causal attn + out-proj) on 8 trn2 cores.

Sharding: 2D batch x head-group. Core c handles batch c//4 and heads
4*(c%4) .. 4*(c%4)+4 (Wq cols + Wo rows). K/V (single MQA head) is computed
redundantly per core for its batch. Out-proj produces per-core partials which
the host reduces (4 partials per batch).

Per-core dataflow (bf16 matmuls, f32 accumulation):
  LN(x) natural layout -> DMA-xbar transpose (SP-dispatched, runs on DMA
  engines) to xn^T -> q^T (4 heads) / k^T / v^T projections -> RoPE on
  q^T,k^T -> per 512-token q-group: S^T = k @ q^T causal-blocked, exp on
  ScalarE (scale folded; |S*scale| small enough to skip max subtraction),
  AV accumulation + row sums via ones-matmul -> normalize with
  reciprocal_approx_fast + partition_broadcast -> out-proj partial.

ScalarE uses only {Ln, Exp, Identity, Copy} which live in ONE activation
table set (natural_log_exp_and_others), so there are no table reloads even
with phases interleaved; rstd = exp(-0.5*ln(var+eps)).
"""

import sys

if "/opt/trn_rl_repo" not in sys.path:
    sys.path.insert(0, "/opt/trn_rl_repo")

import ml_dtypes
import numpy as np

import concourse.bass as bass
import concourse.tile as tile
from concourse import bacc, mybir

F32 = mybir.dt.float32
DT = mybir.dt.bfloat16  # matmul operand storage dtype
DT_NP = ml_dtypes.bfloat16

B, N, DIM, DH, HEADS = 2, 2048, 2048, 128, 16
H_LOCAL = 4  # heads per core
B_GROUPS = 2  # batch split
N_CORES = 8
KT = DIM // 128  # k-tiles over the model dim
TT = N // 128  # token tiles (one batch)
CHUNK = 512  # token chunk for projection phase + q-group width
NCH = N // CHUNK  # chunks per batch
SCALE = float(DH) ** -0.5
EPS = 1e-5
NEG = -1e30


def build_nc(repeat=1):
    nc = bacc.Bacc(None, target_bir_lowering=False, debug=False)

    x_d = nc.dram_tensor("x_in", [N, DIM], DT, kind="ExternalInput")
    wq_d = nc.dram_tensor("wq", [128, KT, H_LOCAL * DH], DT, kind="ExternalInput")
    wk_d = nc.dram_tensor("wk", [128, KT, DH], DT, kind="ExternalInput")
    wv_d = nc.dram_tensor("wv", [128, KT, DH], DT, kind="ExternalInput")
    wo_d = nc.dram_tensor("wo", [128, H_LOCAL, DIM], DT, kind="ExternalInput")
    cos_d = nc.dram_tensor("cosT", [DH, N], F32, kind="ExternalInput")
    sin_d = nc.dram_tensor("sinT", [DH, N], F32, kind="ExternalInput")
    msk_d = nc.dram_tensor("mask", [128, 128], F32, kind="ExternalInput")
    out_d = nc.dram_tensor("out_partial", [N, DIM], DT, kind="ExternalOutput")

    with tile.TileContext(nc) as tc:
        with (
            tc.tile_pool(name="const", bufs=1) as const,
            tc.tile_pool(name="xp", bufs=5) as xp,
            tc.tile_pool(name="xnp", bufs=3) as xnp,
            tc.tile_pool(name="xtp", bufs=2) as xtp,
            tc.tile_pool(name="store", bufs=1) as store,
            tc.tile_pool(name="small", bufs=4) as small,
            tc.tile_pool(name="rope", bufs=4) as ropep,
            tc.tile_pool(name="ep", bufs=3) as ep,
            tc.tile_pool(name="bounce", bufs=2) as bounce,
            tc.tile_pool(name="op", bufs=3) as op,
            tc.tile_pool(name="ps", bufs=1, space="PSUM") as ps,
        ):
            # --- constants ---
            # weights go on the ACT hwdge queue so chunk-0 x loads (SP queue)
            # aren't serialized behind ~20us of weight DMA at kernel start
            wq_sb = const.tile([128, KT, H_LOCAL * DH], DT)
            nc.scalar.dma_start(wq_sb[:], wq_d[:])
            wk_sb = const.tile([128, KT, DH], DT)
            nc.scalar.dma_start(wk_sb[:], wk_d[:])
            wv_sb = const.tile([128, KT, DH], DT)
            nc.scalar.dma_start(wv_sb[:], wv_d[:])
            wo_sb = const.tile([128, H_LOCAL, DIM], DT)
            nc.scalar.dma_start(wo_sb[:], wo_d[:])
            cos_sb = const.tile([DH, N], F32)
            nc.scalar.dma_start(cos_sb[:], cos_d[:])
            sin_sb = const.tile([DH, N], F32)
            nc.scalar.dma_start(sin_sb[:], sin_d[:])
            msk_sb = const.tile([128, 128], F32)
            nc.scalar.dma_start(msk_sb[:], msk_d[:])
            ones_mm = const.tile([128, 1], DT)
            nc.vector.memset(ones_mm, 1.0)

            # --- persistent activations (one batch) ---
            qT_sb = store.tile([DH, H_LOCAL, N], DT, tag="qT")
            kT_sb = store.tile([DH, N], DT, tag="kT")
            v_sb = store.tile([128, TT, DH], DT, tag="v")
            aoT_sb = store.tile([DH, H_LOCAL, N], DT, tag="aoT")

            def rope_evict(dst, src_ps, t0, t1):
                # dst = src*cos + rotate_half(src)*sin_signed, src [128, n] PSUM
                n = t1 - t0
                rot = ropep.tile([DH, CHUNK], DT, tag="rot")
                nc.scalar.copy(rot[0:64, :n], src_ps[64:128, :])
                nc.scalar.copy(rot[64:128, :n], src_ps[0:64, :])
                tmp = ropep.tile([DH, CHUNK], DT, tag="tmp")
                nc.vector.tensor_mul(tmp[:, :n], src_ps[:], cos_sb[:, t0:t1])
                rot2 = ropep.tile([DH, CHUNK], DT, tag="rot2")
                nc.vector.tensor_mul(rot2[:, :n], rot[:, :n], sin_sb[:, t0:t1])
                nc.vector.tensor_add(dst, tmp[:, :n], rot2[:, :n])

            for _rep in range(repeat):
                for cg in range(NCH):
                    c0 = cg * CHUNK
                    # ---- LN stats ----
                    xts = []
                    mr = small.tile([128, 4, 2], F32, tag="mr")
                    for t in range(CHUNK // 128):
                        tok0 = c0 + t * 128
                        x_t = xp.tile([128, DIM], DT, tag="x")
                        xts.append(x_t)
                        nc.sync.dma_start(x_t[:], x_d[tok0 : tok0 + 128, :])
                        stats = small.tile([128, 4, 6], F32, tag="stats")
                        for i in range(4):
                            nc.vector.bn_stats(
                                out=stats[:, i, :], in_=x_t[:, i * 512 : (i + 1) * 512]
                            )
                        nc.vector.bn_aggr(out=mr[:, t, :], in_=stats[:])
                    # rstd = rsqrt(var + eps) on DVE only (bit-trick seed + 2
                    # Newton steps) so ScalarE never needs the sqrt/ln act
                    # tables — Exp stays resident, zero table reloads.
                    v4 = small.tile([128, 4], F32, tag="v4")
                    nc.vector.tensor_scalar(
                        out=v4[:], in0=mr[:, :, 1],
                        scalar1=EPS, scalar2=0.0,
                        op0=mybir.AluOpType.add, op1=mybir.AluOpType.add,
                    )
                    y0 = small.tile([128, 4], F32, tag="y0")
                    nc.vector.tensor_scalar(
                        out=y0[:].bitcast(mybir.dt.int32),
                        in0=v4[:].bitcast(mybir.dt.int32),
                        scalar1=1, scalar2=-1,
                        op0=mybir.AluOpType.logical_shift_right,
                        op1=mybir.AluOpType.bitwise_xor,
                    )
                    nc.vector.tensor_scalar(
                        out=y0[:].bitcast(mybir.dt.int32),
                        in0=y0[:].bitcast(mybir.dt.int32),
                        scalar1=0x5F3759E0, scalar2=0,
                        op0=mybir.AluOpType.add, op1=mybir.AluOpType.add,
                    )
                    rstd4 = y0
                    for _nr in range(2):
                        a = small.tile([128, 4], F32, tag=f"nr{_nr}")
                        nc.vector.tensor_mul(a[:], rstd4[:], rstd4[:])
                        nc.vector.tensor_mul(a[:], a[:], v4[:])
                        nc.vector.tensor_scalar(
                            out=a[:], in0=a[:],
                            scalar1=-0.5, scalar2=1.5,
                            op0=mybir.AluOpType.mult, op1=mybir.AluOpType.add,
                        )
                        yn = small.tile([128, 4], F32, tag=f"y{_nr + 1}")
                        nc.vector.tensor_mul(yn[:], a[:], rstd4[:])
                        rstd4 = yn
                    # ---- LN apply + DMA-xbar transpose ----
                    xnT = xtp.tile([128, KT, CHUNK], DT, tag="xnT")
                    for t in range(CHUNK // 128):
                        x_t = xts[t]
                        xn_t = xnp.tile([128, DIM], DT, tag="xn")
                        if t % 2 == 0:
                            nc.vector.tensor_scalar(
                                out=xn_t[:],
                                in0=x_t[:],
                                scalar1=mr[:, t, 0:1],
                                scalar2=rstd4[:, t : t + 1],
                                op0=mybir.AluOpType.subtract,
                                op1=mybir.AluOpType.mult,
                            )
                        else:
                            negmur = small.tile([128, 1], F32, tag="negmur")
                            nc.vector.tensor_scalar(
                                out=negmur[:],
                                in0=mr[:, t, 0:1],
                                scalar1=rstd4[:, t : t + 1],
                                scalar2=-1.0,
                                op0=mybir.AluOpType.mult,
                                op1=mybir.AluOpType.mult,
                            )
                            nc.scalar.activation(
                                out=xn_t[:],
                                in_=x_t[:],
                                func=mybir.ActivationFunctionType.Identity,
                                bias=negmur[:],
                                scale=rstd4[:, t : t + 1],
                            )
                        nc.sync.dma_start_transpose(
                            xnT[:, :, t * 128 : (t + 1) * 128], xn_t[:]
                        )

                    # ---- projections: 3 sweeps of 2 accumulators ----
                    ktp = ps.tile([DH, CHUNK], F32, tag="acc", bufs=2)
                    vtp = ps.tile([DH, CHUNK], F32, tag="acc", bufs=2)
                    for kt in range(KT):
                        rhs = xnT[:, kt, :]
                        nc.tensor.matmul(
                            ktp[:], wk_sb[:, kt, :], rhs,
                            start=(kt == 0), stop=(kt == KT - 1),
                        )
                        nc.tensor.matmul(
                            vtp[:], wv_sb[:, kt, :], rhs,
                            start=(kt == 0), stop=(kt == KT - 1),
                        )
                    rope_evict(kT_sb[:, c0 : c0 + CHUNK], ktp, c0, c0 + CHUNK)
                    vT_sb = bounce.tile([DH, CHUNK], DT, tag="vT")
                    nc.scalar.copy(vT_sb[:], vtp[:])
                    nc.sync.dma_start_transpose(
                        v_sb[:, cg * 4 : (cg + 1) * 4, :], vT_sb[:]
                    )
                    for hp in range(H_LOCAL // 2):
                        qta = ps.tile([DH, CHUNK], F32, tag="acc", bufs=2)
                        qtb = ps.tile([DH, CHUNK], F32, tag="acc", bufs=2)
                        ha, hb = 2 * hp, 2 * hp + 1
                        for kt in range(KT):
                            rhs = xnT[:, kt, :]
                            nc.tensor.matmul(
                                qta[:], wq_sb[:, kt, ha * DH : (ha + 1) * DH], rhs,
                                start=(kt == 0), stop=(kt == KT - 1),
                            )
                            nc.tensor.matmul(
                                qtb[:], wq_sb[:, kt, hb * DH : (hb + 1) * DH], rhs,
                                start=(kt == 0), stop=(kt == KT - 1),
                            )
                        rope_evict(qT_sb[:, ha, c0 : c0 + CHUNK], qta, c0, c0 + CHUNK)
                        rope_evict(qT_sb[:, hb, c0 : c0 + CHUNK], qtb, c0, c0 + CHUNK)

                    # ---- attention for q-group qg == cg ----
                    nkt = (cg + 1) * (CHUNK // 128)
                    for h in range(H_LOCAL):
                        avT = ps.tile([DH, CHUNK], F32, tag="av", bufs=1)
                        sums = ps.tile([1, CHUNK], F32, tag="sums", bufs=1)
                        for kt in range(nkt):
                            off = max(0, kt * 128 - c0)
                            st = ps.tile([128, CHUNK], F32, tag="s", bufs=2)
                            nc.tensor.matmul(
                                st[:, off:],
                                kT_sb[:, kt * 128 : (kt + 1) * 128],
                                qT_sb[:, h, c0 + off : c0 + CHUNK],
                            )
                            if kt * 128 >= c0:  # diagonal block: causal mask
                                nc.vector.tensor_add(
                                    st[:, off : off + 128],
                                    st[:, off : off + 128],
                                    msk_sb[:],
                                )
                            et = ep.tile([128, CHUNK], DT, tag="et")
                            nc.scalar.activation(
                                out=et[:, off:],
                                in_=st[:, off:],
                                func=mybir.ActivationFunctionType.Exp,
                                scale=SCALE,
                            )
                            nc.tensor.matmul(
                                avT[:, off:],
                                v_sb[:, kt, :],
                                et[:, off:],
                                start=(kt == 0),
                                stop=(kt == nkt - 1),
                            )
                            nc.tensor.matmul(
                                sums[:, off:],
                                ones_mm[:],
                                et[:, off:],
                                start=(kt == 0),
                                stop=(kt == nkt - 1),
                            )
                        recip = small.tile([1, CHUNK], F32, tag="recip")
                        nc.vector.reciprocal_approx_fast(out=recip[:], in_=sums[:])
                        rbc = bounce.tile([128, CHUNK], F32, tag="rbc")
                        nc.gpsimd.partition_broadcast(rbc[:], recip[:])
                        nc.vector.tensor_mul(
                            aoT_sb[:, h, c0 : c0 + CHUNK], avT[:], rbc[:]
                        )

                    # ---- out-proj for this chunk's token tiles ----
                    for tt in range(cg * 4, (cg + 1) * 4):
                        for dg in range(4):
                            opp = ps.tile([128, 512], F32, tag="opp", bufs=2)
                            for h in range(H_LOCAL):
                                nc.tensor.matmul(
                                    opp[:],
                                    aoT_sb[:, h, tt * 128 : (tt + 1) * 128],
                                    wo_sb[:, h, dg * 512 : (dg + 1) * 512],
                                    start=(h == 0),
                                    stop=(h == H_LOCAL - 1),
                                )
                            ot = op.tile([128, 512], DT, tag="ot")
                            if dg % 2 == 0:
                                nc.scalar.copy(ot[:], opp[:])
                            else:
                                nc.vector.tensor_copy(ot[:], opp[:])
                            nc.sync.dma_start(
                                out_d[
                                    tt * 128 : (tt + 1) * 128,
                                    dg * 512 : (dg + 1) * 512,
                                ],
                                ot[:],
                            )

    nc.compile()
    return nc


def make_in_maps(x, gamma, Wq, Wkv, Wo):
    x = np.asarray(x, dtype=np.float32)
    g = np.asarray(gamma, dtype=np.float32)
    Wq = np.asarray(Wq, dtype=np.float32) * g[:, None]
    Wkv = np.asarray(Wkv, dtype=np.float32) * g[:, None]
    Wo = np.asarray(Wo, dtype=np.float32)

    t = np.arange(N, dtype=np.float64)
    inv = 1.0 / (10000.0 ** (np.arange(0, DH, 2, dtype=np.float64) / DH))  # [64]
    fr = np.outer(inv, t)  # [d, t]
    cosT = np.concatenate([np.cos(fr), np.cos(fr)], 0).astype(np.float32)
    sinT = np.concatenate([-np.sin(fr), np.sin(fr)], 0).astype(np.float32)
    mask = np.where(
        np.arange(128)[:, None] > np.arange(128)[None, :], NEG, 0.0
    ).astype(np.float32)

    def pt(w):  # [DIM, M] -> [128, KT, M] partition-major
        return np.ascontiguousarray(
            w.reshape(KT, 128, -1).transpose(1, 0, 2).astype(DT_NP)
        )

    Wk = Wkv[:, :DH]
    Wv = Wkv[:, DH:]
    xb = [np.ascontiguousarray(x[b].astype(DT_NP)) for b in range(B)]
    maps = []
    for c in range(N_CORES):
        b = c // (N_CORES // B_GROUPS)
        hg = c % (N_CORES // B_GROUPS)
        wq_c = pt(Wq[:, hg * H_LOCAL * DH : (hg + 1) * H_LOCAL * DH])
        wo_c = np.ascontiguousarray(
            Wo[hg * H_LOCAL * DH : (hg + 1) * H_LOCAL * DH]
            .reshape(H_LOCAL, DH, DIM)
            .transpose(1, 0, 2)
            .astype(DT_NP)
        )
        maps.append(
            {
                "x_in": xb[b],
                "wq": wq_c,
                "wk": pt(Wk),
                "wv": pt(Wv),
                "wo": wo_c,
                "cosT": cosT,
                "sinT": sinT,
                "mask": mask,
            }
        )
    return maps


_NC_CACHE = {}


def get_nc(repeat=1):
    key = repeat
    if key not in _NC_CACHE:
        _NC_CACHE[key] = build_nc(repeat)
    return _NC_CACHE[key]


def kernel(x, gamma, Wq, Wkv, Wo, _trace=False, _repeat=1):
    from concourse import bass_utils

    nc = get_nc(_repeat)
    in_maps = make_in_maps(x, gamma, Wq, Wkv, Wo)
    res = bass_utils.run_bass_kernel_spmd(
        nc, in_maps, core_ids=list(range(N_CORES)), trace=_trace
    )
    out = np.zeros((B, N, DIM), dtype=np.float32)
    per_b = N_CORES // B_GROUPS
    for c, r in enumerate(res.results):
        out[c // per_b] += np.asarray(r["out_partial"], dtype=np.float32)
    if _trace:
        kernel.last_results = res
    return out


# revision 9
# speedup vs baseline: 1.4995x; 1.1629x over previous
"""MQA attention (LN + QKV proj + RoPE + causal attn + out-proj) on 8 trn2 cores.

Sharding: 2D batch x head-group. Core c handles batch c//4 and heads
4*(c%4) .. 4*(c%4)+4 (Wq cols + Wo rows). K/V (single MQA head) is computed
redundantly per core for its batch. Out-proj produces per-core partials which
the host reduces (4 partials per batch).

Per-core dataflow (bf16 matmuls, f32 accumulation):
  LN(x) natural layout -> DMA-xbar transpose (SP-dispatched, runs on DMA
  engines) to xn^T -> q^T (4 heads) / k^T / v^T projections -> RoPE on
  q^T,k^T -> per 512-token q-group: S^T = k @ q^T causal-blocked, exp on
  ScalarE (scale folded; |S*scale| small enough to skip max subtraction),
  AV accumulation + row sums via ones-matmul -> normalize with
  reciprocal_approx_fast + partition_broadcast -> out-proj partial.

ScalarE uses only {Ln, Exp, Identity, Copy} which live in ONE activation
table set (natural_log_exp_and_others), so there are no table reloads even
with phases interleaved; rstd = exp(-0.5*ln(var+eps)).
"""

import sys

if "/opt/trn_rl_repo" not in sys.path:
    sys.path.insert(0, "/opt/trn_rl_repo")

import ml_dtypes
import numpy as np

import concourse.bass as bass
import concourse.tile as tile
from concourse import bacc, mybir

F32 = mybir.dt.float32
DT = mybir.dt.bfloat16  # matmul operand storage dtype
DT_NP = ml_dtypes.bfloat16

B, N, DIM, DH, HEADS = 2, 2048, 2048, 128, 16
H_LOCAL = 4  # heads per core
B_GROUPS = 2  # batch split
N_CORES = 8
KT = DIM // 128  # k-tiles over the model dim
TT = N // 128  # token tiles (one batch)
CHUNK = 512  # token chunk for projection phase + q-group width
NCH = N // CHUNK  # chunks per batch
SCALE = float(DH) ** -0.5
EPS = 1e-5
NEG = -1e30


def build_nc(repeat=1):
    nc = bacc.Bacc(None, target_bir_lowering=False, debug=False)

    x_d = nc.dram_tensor("x_in", [N, DIM], DT, kind="ExternalInput")
    wq_d = nc.dram_tensor("wq", [128, KT, H_LOCAL * DH], DT, kind="ExternalInput")
    wk_d = nc.dram_tensor("wk", [128, KT, DH], DT, kind="ExternalInput")
    wv_d = nc.dram_tensor("wv", [128, KT, DH], DT, kind="ExternalInput")
    wo_d = nc.dram_tensor("wo", [128, H_LOCAL, DIM], DT, kind="ExternalInput")
    cos_d = nc.dram_tensor("cosT", [DH, N], F32, kind="ExternalInput")
    sin_d = nc.dram_tensor("sinT", [DH, N], F32, kind="ExternalInput")
    msk_d = nc.dram_tensor("mask", [128, 128], F32, kind="ExternalInput")
    out_d = nc.dram_tensor("out_partial", [N, DIM], DT, kind="ExternalOutput")

    with tile.TileContext(nc) as tc:
        with (
            tc.tile_pool(name="const", bufs=1) as const,
            tc.tile_pool(name="xp", bufs=5) as xp,
            tc.tile_pool(name="xnp", bufs=3) as xnp,
            tc.tile_pool(name="xtp", bufs=2) as xtp,
            tc.tile_pool(name="store", bufs=1) as store,
            tc.tile_pool(name="small", bufs=4) as small,
            tc.tile_pool(name="rope", bufs=4) as ropep,
            tc.tile_pool(name="ep", bufs=3) as ep,
            tc.tile_pool(name="bounce", bufs=2) as bounce,
            tc.tile_pool(name="op", bufs=3) as op,
            tc.tile_pool(name="ps", bufs=1, space="PSUM") as ps,
        ):
            # --- constants ---
            # weights go through the idle gpsimd SWDGE queue so chunk-0 x
            # loads (SP hwdge) and LN ops (ACT) are unobstructed at start
            wq_sb = const.tile([128, KT, H_LOCAL * DH], DT)
            nc.gpsimd.dma_start(wq_sb[:], wq_d[:])
            wk_sb = const.tile([128, KT, DH], DT)
            nc.gpsimd.dma_start(wk_sb[:], wk_d[:])
            wv_sb = const.tile([128, KT, DH], DT)
            nc.gpsimd.dma_start(wv_sb[:], wv_d[:])
            wo_sb = const.tile([128, H_LOCAL, DIM], DT)
            nc.gpsimd.dma_start(wo_sb[:], wo_d[:])
            cos_sb = const.tile([DH, N], F32)
            nc.gpsimd.dma_start(cos_sb[:], cos_d[:])
            sin_sb = const.tile([DH, N], F32)
            nc.gpsimd.dma_start(sin_sb[:], sin_d[:])
            msk_sb = const.tile([128, 128], F32)
            nc.gpsimd.dma_start(msk_sb[:], msk_d[:])
            ones_mm = const.tile([128, 1], DT)
            nc.vector.memset(ones_mm, 1.0)

            # --- persistent activations (one batch) ---
            qT_sb = store.tile([DH, H_LOCAL, N], DT, tag="qT")
            kT_sb = store.tile([DH, N], DT, tag="kT")
            v_sb = store.tile([128, TT, DH], DT, tag="v")
            aoT_sb = store.tile([DH, H_LOCAL, N], DT, tag="aoT")

            def rope_evict(dst, src_ps, t0, t1):
                # dst = src*cos + rotate_half(src)*sin_signed, src [128, n] PSUM
                n = t1 - t0
                rot = ropep.tile([DH, CHUNK], DT, tag="rot")
                nc.scalar.copy(rot[0:64, :n], src_ps[64:128, :])
                nc.scalar.copy(rot[64:128, :n], src_ps[0:64, :])
                tmp = ropep.tile([DH, CHUNK], DT, tag="tmp")
                nc.vector.tensor_mul(tmp[:, :n], src_ps[:], cos_sb[:, t0:t1])
                rot2 = ropep.tile([DH, CHUNK], DT, tag="rot2")
                nc.vector.tensor_mul(rot2[:, :n], rot[:, :n], sin_sb[:, t0:t1])
                nc.vector.tensor_add(dst, tmp[:, :n], rot2[:, :n])

            for _rep in range(repeat):
                for cg in range(NCH):
                    c0 = cg * CHUNK
                    # ---- LN stats + apply + transpose, pipelined per pair of
                    # 128-token tiles so the first transposes start early ----
                    mr = small.tile([128, 4, 2], F32, tag="mr")
                    xnT = xtp.tile([128, KT, CHUNK], DT, tag="xnT")
                    for half in range(2):
                        xts = []
                        for i in range(2):
                            t = half * 2 + i
                            tok0 = c0 + t * 128
                            x_t = xp.tile([128, DIM], DT, tag="x")
                            xts.append(x_t)
                            nc.sync.dma_start(x_t[:], x_d[tok0 : tok0 + 128, :])
                            stats = small.tile([128, 4, 6], F32, tag="stats")
                            for w in range(4):
                                nc.vector.bn_stats(
                                    out=stats[:, w, :],
                                    in_=x_t[:, w * 512 : (w + 1) * 512],
                                )
                            nc.vector.bn_aggr(out=mr[:, t, :], in_=stats[:])
                        # rstd = rsqrt(var+eps) on DVE only (bit-trick seed +
                        # 2 Newton steps): ScalarE never needs sqrt/ln act
                        # tables — Exp stays resident, zero table reloads.
                        mrh = mr[:, half * 2 : half * 2 + 2, :]
                        v2t = small.tile([128, 2], F32, tag="v2t")
                        nc.vector.tensor_scalar(
                            out=v2t[:], in0=mrh[:, :, 1],
                            scalar1=EPS, scalar2=0.0,
                            op0=mybir.AluOpType.add, op1=mybir.AluOpType.add,
                        )
                        y0 = small.tile([128, 2], F32, tag="y0")
                        nc.vector.tensor_scalar(
                            out=y0[:].bitcast(mybir.dt.int32),
                            in0=v2t[:].bitcast(mybir.dt.int32),
                            scalar1=1, scalar2=-1,
                            op0=mybir.AluOpType.logical_shift_right,
                            op1=mybir.AluOpType.bitwise_xor,
                        )
                        nc.vector.tensor_scalar(
                            out=y0[:].bitcast(mybir.dt.int32),
                            in0=y0[:].bitcast(mybir.dt.int32),
                            scalar1=0x5F3759E0, scalar2=0,
                            op0=mybir.AluOpType.add, op1=mybir.AluOpType.add,
                        )
                        rstd2 = y0
                        for _nr in range(2):
                            a = small.tile([128, 2], F32, tag=f"nr{_nr}")
                            nc.vector.tensor_mul(a[:], rstd2[:], rstd2[:])
                            nc.vector.tensor_mul(a[:], a[:], v2t[:])
                            nc.vector.tensor_scalar(
                                out=a[:], in0=a[:],
                                scalar1=-0.5, scalar2=1.5,
                                op0=mybir.AluOpType.mult, op1=mybir.AluOpType.add,
                            )
                            yn = small.tile([128, 2], F32, tag=f"y{_nr + 1}")
                            nc.vector.tensor_mul(yn[:], a[:], rstd2[:])
                            rstd2 = yn
                        for i in range(2):
                            t = half * 2 + i
                            x_t = xts[i]
                            xn_t = xnp.tile([128, DIM], DT, tag="xn")
                            if t % 2 == 0:
                                nc.vector.tensor_scalar(
                                    out=xn_t[:],
                                    in0=x_t[:],
                                    scalar1=mr[:, t, 0:1],
                                    scalar2=rstd2[:, i : i + 1],
                                    op0=mybir.AluOpType.subtract,
                                    op1=mybir.AluOpType.mult,
                                )
                            else:
                                negmur = small.tile([128, 1], F32, tag="negmur")
                                nc.vector.tensor_scalar(
                                    out=negmur[:],
                                    in0=mr[:, t, 0:1],
                                    scalar1=rstd2[:, i : i + 1],
                                    scalar2=-1.0,
                                    op0=mybir.AluOpType.mult,
                                    op1=mybir.AluOpType.mult,
                                )
                                nc.scalar.activation(
                                    out=xn_t[:],
                                    in_=x_t[:],
                                    func=mybir.ActivationFunctionType.Identity,
                                    bias=negmur[:],
                                    scale=rstd2[:, i : i + 1],
                                )
                            nc.sync.dma_start_transpose(
                                xnT[:, :, t * 128 : (t + 1) * 128], xn_t[:]
                            )

                    # ---- projections: 3 sweeps of 2 accumulators ----
                    ktp = ps.tile([DH, CHUNK], F32, tag="acc", bufs=2)
                    vtp = ps.tile([DH, CHUNK], F32, tag="acc", bufs=2)
                    for kt in range(KT):
                        rhs = xnT[:, kt, :]
                        nc.tensor.matmul(
                            ktp[:], wk_sb[:, kt, :], rhs,
                            start=(kt == 0), stop=(kt == KT - 1),
                        )
                        nc.tensor.matmul(
                            vtp[:], wv_sb[:, kt, :], rhs,
                            start=(kt == 0), stop=(kt == KT - 1),
                        )
                    rope_evict(kT_sb[:, c0 : c0 + CHUNK], ktp, c0, c0 + CHUNK)
                    vT_sb = bounce.tile([DH, CHUNK], DT, tag="vT")
                    nc.scalar.copy(vT_sb[:], vtp[:])
                    nc.sync.dma_start_transpose(
                        v_sb[:, cg * 4 : (cg + 1) * 4, :], vT_sb[:]
                    )
                    for hp in range(H_LOCAL // 2):
                        qta = ps.tile([DH, CHUNK], F32, tag="acc", bufs=2)
                        qtb = ps.tile([DH, CHUNK], F32, tag="acc", bufs=2)
                        ha, hb = 2 * hp, 2 * hp + 1
                        for kt in range(KT):
                            rhs = xnT[:, kt, :]
                            nc.tensor.matmul(
                                qta[:], wq_sb[:, kt, ha * DH : (ha + 1) * DH], rhs,
                                start=(kt == 0), stop=(kt == KT - 1),
                            )
                            nc.tensor.matmul(
                                qtb[:], wq_sb[:, kt, hb * DH : (hb + 1) * DH], rhs,
                                start=(kt == 0), stop=(kt == KT - 1),
                            )
                        rope_evict(qT_sb[:, ha, c0 : c0 + CHUNK], qta, c0, c0 + CHUNK)
                        rope_evict(qT_sb[:, hb, c0 : c0 + CHUNK], qtb, c0, c0 + CHUNK)

                    # ---- attention for q-group qg == cg ----
                    nkt = (cg + 1) * (CHUNK // 128)
                    for h in range(H_LOCAL):
                        avT = ps.tile([DH, CHUNK], F32, tag="av", bufs=1)
                        # sums shares the out-proj bank ring (same shape/tag):
                        # attention and out-proj never overlap in PE order, and
                        # the freed bank pays for a 3rd S tile below.
                        sums = ps.tile([128, CHUNK], F32, tag="opp", bufs=2)
                        ets = {}

                        def av_sums(kt, h=h, avT=avT, sums=sums):
                            off = max(0, kt * 128 - c0)
                            et = ets.pop(kt)
                            nc.tensor.matmul(
                                avT[:, off:],
                                v_sb[:, kt, :],
                                et[:, off:],
                                start=(kt == 0),
                                stop=(kt == nkt - 1),
                            )
                            nc.tensor.matmul(
                                sums[0:1, off:],
                                ones_mm[:],
                                et[:, off:],
                                start=(kt == 0),
                                stop=(kt == nkt - 1),
                            )

                        # S/exp run 2 k-tiles ahead of AV/sums so the PE never
                        # waits on ScalarE's exp (3 S bufs in flight).
                        for kt in range(nkt):
                            off = max(0, kt * 128 - c0)
                            st = ps.tile([128, CHUNK], F32, tag="s", bufs=3)
                            nc.tensor.matmul(
                                st[:, off:],
                                kT_sb[:, kt * 128 : (kt + 1) * 128],
                                qT_sb[:, h, c0 + off : c0 + CHUNK],
                            )
                            if kt * 128 >= c0:  # diagonal block: causal mask
                                nc.vector.tensor_add(
                                    st[:, off : off + 128],
                                    st[:, off : off + 128],
                                    msk_sb[:],
                                )
                            et = ep.tile([128, CHUNK], DT, tag="et", bufs=4)
                            nc.scalar.activation(
                                out=et[:, off:],
                                in_=st[:, off:],
                                func=mybir.ActivationFunctionType.Exp,
                                scale=SCALE,
                            )
                            ets[kt] = et
                            if kt >= 2:
                                av_sums(kt - 2)
                        for kt in range(max(0, nkt - 2), nkt):
                            av_sums(kt)
                        recip = small.tile([1, CHUNK], F32, tag="recip")
                        nc.vector.reciprocal_approx_fast(
                            out=recip[:], in_=sums[0:1, :]
                        )
                        rbc = bounce.tile([128, CHUNK], F32, tag="rbc")
                        nc.gpsimd.partition_broadcast(rbc[:], recip[:])
                        nc.vector.tensor_mul(
                            aoT_sb[:, h, c0 : c0 + CHUNK], avT[:], rbc[:]
                        )

                    # ---- out-proj for this chunk's token tiles ----
                    for tt in range(cg * 4, (cg + 1) * 4):
                        for dg in range(4):
                            opp = ps.tile([128, 512], F32, tag="opp", bufs=2)
                            for h in range(H_LOCAL):
                                nc.tensor.matmul(
                                    opp[:],
                                    aoT_sb[:, h, tt * 128 : (tt + 1) * 128],
                                    wo_sb[:, h, dg * 512 : (dg + 1) * 512],
                                    start=(h == 0),
                                    stop=(h == H_LOCAL - 1),
                                )
                            ot = op.tile([128, 512], DT, tag="ot")
                            if dg % 2 == 0:
                                nc.scalar.copy(ot[:], opp[:])
                            else:
                                nc.vector.tensor_copy(ot[:], opp[:])
                            nc.sync.dma_start(
                                out_d[
                                    tt * 128 : (tt + 1) * 128,
                                    dg * 512 : (dg + 1) * 512,
                                ],
                                ot[:],
                            )

    nc.compile()
    return nc


def make_in_maps(x, gamma, Wq, Wkv, Wo):
    x = np.asarray(x, dtype=np.float32)
    g = np.asarray(gamma, dtype=np.float32)
    Wq = np.asarray(Wq, dtype=np.float32) * g[:, None]
    Wkv = np.asarray(Wkv, dtype=np.float32) * g[:, None]
    Wo = np.asarray(Wo, dtype=np.float32)

    t = np.arange(N, dtype=np.float64)
    inv = 1.0 / (10000.0 ** (np.arange(0, DH, 2, dtype=np.float64) / DH))  # [64]
    fr = np.outer(inv, t)  # [d, t]
    cosT = np.concatenate([np.cos(fr), np.cos(fr)], 0).astype(np.float32)
    sinT = np.concatenate([-np.sin(fr), np.sin(fr)], 0).astype(np.float32)
    mask = np.where(
        np.arange(128)[:, None] > np.arange(128)[None, :], NEG, 0.0
    ).astype(np.float32)

    def pt(w):  # [DIM, M] -> [128, KT, M] partition-major
        return np.ascontiguousarray(
            w.reshape(KT, 128, -1).transpose(1, 0, 2).astype(DT_NP)
        )

    Wk = Wkv[:, :DH]
    Wv = Wkv[:, DH:]
    xb = [np.ascontiguousarray(x[b].astype(DT_NP)) for b in range(B)]
    maps = []
    for c in range(N_CORES):
        b = c // (N_CORES // B_GROUPS)
        hg = c % (N_CORES // B_GROUPS)
        wq_c = pt(Wq[:, hg * H_LOCAL * DH : (hg + 1) * H_LOCAL * DH])
        wo_c = np.ascontiguousarray(
            Wo[hg * H_LOCAL * DH : (hg + 1) * H_LOCAL * DH]
            .reshape(H_LOCAL, DH, DIM)
            .transpose(1, 0, 2)
            .astype(DT_NP)
        )
        maps.append(
            {
                "x_in": xb[b],
                "wq": wq_c,
                "wk": pt(Wk),
                "wv": pt(Wv),
                "wo": wo_c,
                "cosT": cosT,
                "sinT": sinT,
                "mask": mask,
            }
        )
    return maps


_NC_CACHE = {}


def get_nc(repeat=1):
    key = repeat
    if key not in _NC_CACHE:
        _NC_CACHE[key] = build_nc(repeat)
    return _NC_CACHE[key]


def kernel(x, gamma, Wq, Wkv, Wo, _trace=False, _repeat=1):
    from concourse import bass_utils

    nc = get_nc(_repeat)
    in_maps = make_in_maps(x, gamma, Wq, Wkv, Wo)
    res = bass_utils.run_bass_kernel_spmd(
        nc, in_maps, core_ids=list(range(N_CORES)), trace=_trace
    )
    out = np.zeros((B, N, DIM), dtype=np.float32)
    per_b = N_CORES // B_GROUPS
    for c, r in enumerate(res.results):
        out[c // per_b] += np.asarray(r["out_partial"], dtype=np.float32)
    if _trace:
        kernel.last_results = res
    return out


# revision 17
# speedup vs baseline: 1.6155x; 1.0774x over previous
"""MQA attention (LN + QKV proj + RoPE + causal attn + out-proj) on 8 trn2 cores.

Sharding: 2D batch x head-group. Core c handles batch c//4 and heads
4*(c%4) .. 4*(c%4)+4 (Wq cols + Wo rows). K/V (single MQA head) is computed
redundantly per core for its batch. Out-proj produces per-core partials which
the host reduces (4 partials per batch).

Per-core dataflow (bf16 matmuls, f32 accumulation):
  LN(x) natural layout -> DMA-xbar transpose (SP-dispatched, runs on DMA
  engines) to xn^T -> q^T (4 heads) / k^T / v^T projections -> RoPE on
  q^T,k^T -> per 512-token q-group: S^T = k @ q^T causal-blocked, exp on
  ScalarE (scale folded; |S*scale| small enough to skip max subtraction),
  AV accumulation + row sums via ones-matmul -> normalize with
  reciprocal_approx_fast + partition_broadcast -> out-proj partial.

ScalarE uses only {Ln, Exp, Identity, Copy} which live in ONE activation
table set (natural_log_exp_and_others), so there are no table reloads even
with phases interleaved; rstd = exp(-0.5*ln(var+eps)).
"""

import sys

if "/opt/trn_rl_repo" not in sys.path:
    sys.path.insert(0, "/opt/trn_rl_repo")

import ml_dtypes
import numpy as np

import concourse.bass as bass
import concourse.tile as tile
from concourse import bacc, mybir

F32 = mybir.dt.float32
DT = mybir.dt.bfloat16  # matmul operand storage dtype
DT_NP = ml_dtypes.bfloat16

B, N, DIM, DH, HEADS = 2, 2048, 2048, 128, 16
H_LOCAL = 4  # heads per core
B_GROUPS = 2  # batch split
N_CORES = 8
KT = DIM // 128  # k-tiles over the model dim
TT = N // 128  # token tiles (one batch)
CHUNK = 512  # token chunk for projection phase + q-group width
NCH = N // CHUNK  # chunks per batch
SCALE = float(DH) ** -0.5
EPS = 1e-5
NEG = -1e30


def build_nc(repeat=1):
    nc = bacc.Bacc(None, target_bir_lowering=False, debug=False)

    x_d = nc.dram_tensor("x_in", [N, DIM], DT, kind="ExternalInput")
    wq_d = nc.dram_tensor("wq", [128, KT, H_LOCAL * DH], DT, kind="ExternalInput")
    wk_d = nc.dram_tensor("wk", [128, KT, DH], DT, kind="ExternalInput")
    wv_d = nc.dram_tensor("wv", [128, KT, DH], DT, kind="ExternalInput")
    wo_d = nc.dram_tensor("wo", [128, H_LOCAL, DIM], DT, kind="ExternalInput")
    cos_d = nc.dram_tensor("cosT", [DH, N], DT, kind="ExternalInput")
    sin_d = nc.dram_tensor("sinT", [DH, N], DT, kind="ExternalInput")
    msk_d = nc.dram_tensor("mask", [128, 128], F32, kind="ExternalInput")
    out_d = nc.dram_tensor("out_partial", [N, DIM], DT, kind="ExternalOutput")

    with tile.TileContext(nc) as tc:
        with (
            tc.tile_pool(name="const", bufs=1) as const,
            tc.tile_pool(name="xp", bufs=5) as xp,
            tc.tile_pool(name="xnp", bufs=3) as xnp,
            tc.tile_pool(name="xtp", bufs=2) as xtp,
            tc.tile_pool(name="store", bufs=1) as store,
            tc.tile_pool(name="small", bufs=4) as small,
            tc.tile_pool(name="rope", bufs=4) as ropep,
            tc.tile_pool(name="ep", bufs=3) as ep,
            tc.tile_pool(name="bounce", bufs=2) as bounce,
            tc.tile_pool(name="op", bufs=3) as op,
            tc.tile_pool(name="ps", bufs=1, space="PSUM") as ps,
        ):
            # --- constants ---
            # weights go through the idle gpsimd SWDGE queue so chunk-0 x
            # loads (SP hwdge) and LN ops (ACT) are unobstructed at start.
            # Only k/v weights + rope tables + mask are loaded immediately;
            # wq/wo DMAs are emitted later (when first needed) so they don't
            # steal DMA-engine bandwidth from chunk-0 x tiles.
            wq_sb = const.tile([128, KT, H_LOCAL * DH], DT)
            wk_sb = const.tile([128, KT, DH], DT)
            nc.gpsimd.dma_start(wk_sb[:], wk_d[:])
            wv_sb = const.tile([128, KT, DH], DT)
            nc.gpsimd.dma_start(wv_sb[:], wv_d[:])
            wo_sb = const.tile([128, H_LOCAL, DIM], DT)
            cos_sb = const.tile([DH, N], DT)
            nc.gpsimd.dma_start(cos_sb[:], cos_d[:])
            sin_sb = const.tile([DH, N], DT)
            nc.gpsimd.dma_start(sin_sb[:], sin_d[:])
            msk_sb = const.tile([128, 128], F32)
            nc.gpsimd.dma_start(msk_sb[:], msk_d[:])
            ones_mm = const.tile([128, 1], DT)
            nc.vector.memset(ones_mm, 1.0)

            # --- persistent activations (one batch) ---
            qT_sb = store.tile([DH, H_LOCAL, N], DT, tag="qT")
            kT_sb = store.tile([DH, N], DT, tag="kT")
            v_sb = store.tile([128, TT, DH], DT, tag="v")
            aoT_sb = store.tile([DH, H_LOCAL, N], DT, tag="aoT")

            def rope_evict(dst, src_ps, t0, t1):
                # dst = src*cos + rotate_half(src)*sin_signed, src [128, n] PSUM
                n = t1 - t0
                rot = ropep.tile([DH, CHUNK], DT, tag="rot")
                nc.scalar.copy(rot[0:64, :n], src_ps[64:128, :])
                nc.scalar.copy(rot[64:128, :n], src_ps[0:64, :])
                tmp = ropep.tile([DH, CHUNK], DT, tag="tmp")
                nc.vector.tensor_mul(tmp[:, :n], src_ps[:], cos_sb[:, t0:t1])
                rot2 = ropep.tile([DH, CHUNK], DT, tag="rot2")
                nc.vector.tensor_mul(rot2[:, :n], rot[:, :n], sin_sb[:, t0:t1])
                nc.vector.tensor_add(dst, tmp[:, :n], rot2[:, :n])

            def emit_outproj(ocg):
                for tt in range(ocg * 4, (ocg + 1) * 4):
                    for dg in range(4):
                        opp = ps.tile([128, 512], F32, tag="opp", bufs=2)
                        for h in range(H_LOCAL):
                            nc.tensor.matmul(
                                opp[:],
                                aoT_sb[:, h, tt * 128 : (tt + 1) * 128],
                                wo_sb[:, h, dg * 512 : (dg + 1) * 512],
                                start=(h == 0),
                                stop=(h == H_LOCAL - 1),
                            )
                        ot = op.tile([128, 512], DT, tag="ot")
                        if dg % 2 == 0:
                            nc.scalar.copy(ot[:], opp[:])
                        else:
                            nc.vector.tensor_copy(ot[:], opp[:])
                        nc.sync.dma_start(
                            out_d[
                                tt * 128 : (tt + 1) * 128,
                                dg * 512 : (dg + 1) * 512,
                            ],
                            ot[:],
                        )

            for _rep in range(repeat):
                for cg in range(NCH):
                    c0 = cg * CHUNK
                    # ---- LN stats + apply + transpose, pipelined per pair of
                    # 128-token tiles so the first transposes start early ----
                    mr = small.tile([128, 4, 2], F32, tag="mr")
                    xnT = xtp.tile([128, KT, CHUNK], DT, tag="xnT")
                    for half in range(2):
                        xts = []
                        for i in range(2):
                            t = half * 2 + i
                            tok0 = c0 + t * 128
                            x_t = xp.tile([128, DIM], DT, tag="x")
                            xts.append(x_t)
                            nc.sync.dma_start(x_t[:], x_d[tok0 : tok0 + 128, :])
                            stats = small.tile([128, 4, 6], F32, tag="stats")
                            for w in range(4):
                                nc.vector.bn_stats(
                                    out=stats[:, w, :],
                                    in_=x_t[:, w * 512 : (w + 1) * 512],
                                )
                            nc.vector.bn_aggr(out=mr[:, t, :], in_=stats[:])
                        # rstd = rsqrt(var+eps) on DVE only (bit-trick seed +
                        # 2 Newton steps): ScalarE never needs sqrt/ln act
                        # tables — Exp stays resident, zero table reloads.
                        mrh = mr[:, half * 2 : half * 2 + 2, :]
                        v2t = small.tile([128, 2], F32, tag="v2t")
                        nc.vector.tensor_scalar(
                            out=v2t[:], in0=mrh[:, :, 1],
                            scalar1=EPS, scalar2=0.0,
                            op0=mybir.AluOpType.add, op1=mybir.AluOpType.add,
                        )
                        y0 = small.tile([128, 2], F32, tag="y0")
                        nc.vector.tensor_scalar(
                            out=y0[:].bitcast(mybir.dt.int32),
                            in0=v2t[:].bitcast(mybir.dt.int32),
                            scalar1=1, scalar2=-1,
                            op0=mybir.AluOpType.logical_shift_right,
                            op1=mybir.AluOpType.bitwise_xor,
                        )
                        nc.vector.tensor_scalar(
                            out=y0[:].bitcast(mybir.dt.int32),
                            in0=y0[:].bitcast(mybir.dt.int32),
                            scalar1=0x5F3759E0, scalar2=0,
                            op0=mybir.AluOpType.add, op1=mybir.AluOpType.add,
                        )
                        rstd2 = y0
                        for _nr in range(2):
                            a = small.tile([128, 2], F32, tag=f"nr{_nr}")
                            nc.vector.tensor_mul(a[:], rstd2[:], rstd2[:])
                            nc.vector.tensor_mul(a[:], a[:], v2t[:])
                            nc.vector.tensor_scalar(
                                out=a[:], in0=a[:],
                                scalar1=-0.5, scalar2=1.5,
                                op0=mybir.AluOpType.mult, op1=mybir.AluOpType.add,
                            )
                            yn = small.tile([128, 2], F32, tag=f"y{_nr + 1}")
                            nc.vector.tensor_mul(yn[:], a[:], rstd2[:])
                            rstd2 = yn
                        for i in range(2):
                            t = half * 2 + i
                            x_t = xts[i]
                            xn_t = xnp.tile([128, DIM], DT, tag="xn")
                            if t % 2 == 0:
                                nc.vector.tensor_scalar(
                                    out=xn_t[:],
                                    in0=x_t[:],
                                    scalar1=mr[:, t, 0:1],
                                    scalar2=rstd2[:, i : i + 1],
                                    op0=mybir.AluOpType.subtract,
                                    op1=mybir.AluOpType.mult,
                                )
                            else:
                                negmur = small.tile([128, 1], F32, tag="negmur")
                                nc.vector.tensor_scalar(
                                    out=negmur[:],
                                    in0=mr[:, t, 0:1],
                                    scalar1=rstd2[:, i : i + 1],
                                    scalar2=-1.0,
                                    op0=mybir.AluOpType.mult,
                                    op1=mybir.AluOpType.mult,
                                )
                                nc.scalar.activation(
                                    out=xn_t[:],
                                    in_=x_t[:],
                                    func=mybir.ActivationFunctionType.Identity,
                                    bias=negmur[:],
                                    scale=rstd2[:, i : i + 1],
                                )
                            nc.sync.dma_start_transpose(
                                xnT[:, :, t * 128 : (t + 1) * 128], xn_t[:]
                            )

                    # ---- projections: 3 sweeps of 2 accumulators ----
                    if _rep == 0 and cg == 0:
                        # wq arrives while the k/v sweep runs; wo later still
                        nc.gpsimd.dma_start(wq_sb[:], wq_d[:])
                    ktp = ps.tile([DH, CHUNK], F32, tag="acc", bufs=2)
                    vtp = ps.tile([DH, CHUNK], F32, tag="acc", bufs=2)
                    for kt in range(KT):
                        rhs = xnT[:, kt, :]
                        nc.tensor.matmul(
                            ktp[:], wk_sb[:, kt, :], rhs,
                            start=(kt == 0), stop=(kt == KT - 1),
                        )
                        nc.tensor.matmul(
                            vtp[:], wv_sb[:, kt, :], rhs,
                            start=(kt == 0), stop=(kt == KT - 1),
                        )
                    rope_evict(kT_sb[:, c0 : c0 + CHUNK], ktp, c0, c0 + CHUNK)
                    vT_sb = bounce.tile([DH, CHUNK], DT, tag="vT")
                    nc.scalar.copy(vT_sb[:], vtp[:])
                    nc.sync.dma_start_transpose(
                        v_sb[:, cg * 4 : (cg + 1) * 4, :], vT_sb[:]
                    )
                    # out-proj of the PREVIOUS chunk goes here: its matmuls
                    # fill the PE stalls at proj-sweep boundaries (the acc
                    # banks drain through rope-evict between sweeps).
                    if cg > 0:
                        emit_outproj(cg - 1)
                    for hp in range(H_LOCAL // 2):
                        qta = ps.tile([DH, CHUNK], F32, tag="acc", bufs=2)
                        qtb = ps.tile([DH, CHUNK], F32, tag="acc", bufs=2)
                        ha, hb = 2 * hp, 2 * hp + 1
                        for kt in range(KT):
                            rhs = xnT[:, kt, :]
                            nc.tensor.matmul(
                                qta[:], wq_sb[:, kt, ha * DH : (ha + 1) * DH], rhs,
                                start=(kt == 0), stop=(kt == KT - 1),
                            )
                            nc.tensor.matmul(
                                qtb[:], wq_sb[:, kt, hb * DH : (hb + 1) * DH], rhs,
                                start=(kt == 0), stop=(kt == KT - 1),
                            )
                        rope_evict(qT_sb[:, ha, c0 : c0 + CHUNK], qta, c0, c0 + CHUNK)
                        rope_evict(qT_sb[:, hb, c0 : c0 + CHUNK], qtb, c0, c0 + CHUNK)

                    # ---- attention for q-group qg == cg ----
                    nkt = (cg + 1) * (CHUNK // 128)
                    for h in range(H_LOCAL):
                        avT = ps.tile([DH, CHUNK], F32, tag="av", bufs=1)
                        # sums shares the out-proj bank ring (same shape/tag):
                        # attention and out-proj never overlap in PE order, and
                        # the freed bank pays for a 3rd S tile below.
                        sums = ps.tile([128, CHUNK], F32, tag="opp", bufs=2)
                        ets = {}

                        def av_sums(kt, h=h, avT=avT, sums=sums):
                            off = max(0, kt * 128 - c0)
                            et = ets.pop(kt)
                            nc.tensor.matmul(
                                avT[:, off:],
                                v_sb[:, kt, :],
                                et[:, off:],
                                start=(kt == 0),
                                stop=(kt == nkt - 1),
                            )
                            nc.tensor.matmul(
                                sums[0:1, off:],
                                ones_mm[:],
                                et[:, off:],
                                start=(kt == 0),
                                stop=(kt == nkt - 1),
                            )

                        # S/exp run 2 k-tiles ahead of AV/sums so the PE never
                        # waits on ScalarE's exp (3 S bufs in flight).
                        for kt in range(nkt):
                            off = max(0, kt * 128 - c0)
                            st = ps.tile([128, CHUNK], F32, tag="s", bufs=3)
                            nc.tensor.matmul(
                                st[:, off:],
                                kT_sb[:, kt * 128 : (kt + 1) * 128],
                                qT_sb[:, h, c0 + off : c0 + CHUNK],
                            )
                            if kt * 128 >= c0:  # diagonal block: causal mask
                                nc.vector.tensor_add(
                                    st[:, off : off + 128],
                                    st[:, off : off + 128],
                                    msk_sb[:],
                                )
                            et = ep.tile([128, CHUNK], DT, tag="et", bufs=4)
                            nc.scalar.activation(
                                out=et[:, off:],
                                in_=st[:, off:],
                                func=mybir.ActivationFunctionType.Exp,
                                scale=SCALE,
                            )
                            ets[kt] = et
                            if kt >= 2:
                                av_sums(kt - 2)
                        for kt in range(max(0, nkt - 2), nkt):
                            av_sums(kt)
                        recip = small.tile([1, CHUNK], F32, tag="recip")
                        nc.vector.reciprocal_approx_fast(
                            out=recip[:], in_=sums[0:1, :]
                        )
                        rbc = bounce.tile([128, CHUNK], F32, tag="rbc")
                        nc.gpsimd.partition_broadcast(rbc[:], recip[:])
                        nc.vector.tensor_mul(
                            aoT_sb[:, h, c0 : c0 + CHUNK], avT[:], rbc[:]
                        )
                    if _rep == 0 and cg == 0:
                        nc.gpsimd.dma_start(wo_sb[:], wo_d[:])

                # out-proj of the final chunk has no later sweep to hide in
                emit_outproj(NCH - 1)

    nc.compile()
    return nc


def make_in_maps(x, gamma, Wq, Wkv, Wo):
    x = np.asarray(x, dtype=np.float32)
    g = np.asarray(gamma, dtype=np.float32)
    Wq = np.asarray(Wq, dtype=np.float32) * g[:, None]
    Wkv = np.asarray(Wkv, dtype=np.float32) * g[:, None]
    Wo = np.asarray(Wo, dtype=np.float32)

    t = np.arange(N, dtype=np.float64)
    inv = 1.0 / (10000.0 ** (np.arange(0, DH, 2, dtype=np.float64) / DH))  # [64]
    fr = np.outer(inv, t)  # [d, t]
    cosT = np.ascontiguousarray(
        np.concatenate([np.cos(fr), np.cos(fr)], 0).astype(DT_NP)
    )
    sinT = np.ascontiguousarray(
        np.concatenate([-np.sin(fr), np.sin(fr)], 0).astype(DT_NP)
    )
    mask = np.where(
        np.arange(128)[:, None] > np.arange(128)[None, :], NEG, 0.0
    ).astype(np.float32)

    def pt(w):  # [DIM, M] -> [128, KT, M] partition-major
        return np.ascontiguousarray(
            w.reshape(KT, 128, -1).transpose(1, 0, 2).astype(DT_NP)
        )

    Wk = Wkv[:, :DH]
    Wv = Wkv[:, DH:]
    xb = [np.ascontiguousarray(x[b].astype(DT_NP)) for b in range(B)]
    maps = []
    for c in range(N_CORES):
        b = c // (N_CORES // B_GROUPS)
        hg = c % (N_CORES // B_GROUPS)
        wq_c = pt(Wq[:, hg * H_LOCAL * DH : (hg + 1) * H_LOCAL * DH])
        wo_c = np.ascontiguousarray(
            Wo[hg * H_LOCAL * DH : (hg + 1) * H_LOCAL * DH]
            .reshape(H_LOCAL, DH, DIM)
            .transpose(1, 0, 2)
            .astype(DT_NP)
        )
        maps.append(
            {
                "x_in": xb[b],
                "wq": wq_c,
                "wk": pt(Wk),
                "wv": pt(Wv),
                "wo": wo_c,
                "cosT": cosT,
                "sinT": sinT,
                "mask": mask,
            }
        )
    return maps


_NC_CACHE = {}


def get_nc(repeat=1):
    key = repeat
    if key not in _NC_CACHE:
        _NC_CACHE[key] = build_nc(repeat)
    return _NC_CACHE[key]


def kernel(x, gamma, Wq, Wkv, Wo, _trace=False, _repeat=1):
    from concourse import bass_utils

    nc = get_nc(_repeat)
    in_maps = make_in_maps(x, gamma, Wq, Wkv, Wo)
    res = bass_utils.run_bass_kernel_spmd(
        nc, in_maps, core_ids=list(range(N_CORES)), trace=_trace
    )
    out = np.zeros((B, N, DIM), dtype=np.float32)
    per_b = N_CORES // B_GROUPS
    for c, r in enumerate(res.results):
        out[c // per_b] += np.asarray(r["out_partial"], dtype=np.float32)
    if _trace:
        kernel.last_results = res
    return out


# revision 19
# speedup vs baseline: 1.7679x; 1.0943x over previous
"""MQA attention (LN + QKV proj + RoPE + causal attn + out-proj) on 8 trn2 cores.

Sharding: 2D batch x head-group. Core c handles batch c//4 and heads
4*(c%4) .. 4*(c%4)+4 (Wq cols + Wo rows). K/V (single MQA head) is computed
redundantly per core for its batch. Out-proj produces per-core partials which
the host reduces (4 partials per batch).

Per-core dataflow (bf16 matmuls, f32 accumulation):
  LN(x) natural layout -> DMA-xbar transpose (SP-dispatched, runs on DMA
  engines) to xn^T -> q^T (4 heads) / k^T / v^T projections -> RoPE on
  q^T,k^T -> per 512-token q-group: S^T = k @ q^T causal-blocked, exp on
  ScalarE (scale folded; |S*scale| small enough to skip max subtraction),
  AV accumulation + row sums via ones-matmul -> normalize with
  reciprocal_approx_fast + partition_broadcast -> out-proj partial.

ScalarE uses only {Ln, Exp, Identity, Copy} which live in ONE activation
table set (natural_log_exp_and_others), so there are no table reloads even
with phases interleaved; rstd = exp(-0.5*ln(var+eps)).
"""

import sys

if "/opt/trn_rl_repo" not in sys.path:
    sys.path.insert(0, "/opt/trn_rl_repo")

import ml_dtypes
import numpy as np

import concourse.bass as bass
import concourse.tile as tile
from concourse import bacc, mybir

F32 = mybir.dt.float32
DT = mybir.dt.bfloat16  # matmul operand storage dtype
DT_NP = ml_dtypes.bfloat16

B, N, DIM, DH, HEADS = 2, 2048, 2048, 128, 16
H_LOCAL = 4  # heads per core
B_GROUPS = 2  # batch split
N_CORES = 8
KT = DIM // 128  # k-tiles over the model dim
TT = N // 128  # token tiles (one batch)
CHUNK = 512  # token chunk for projection phase + q-group width
NCH = N // CHUNK  # chunks per batch
SCALE = float(DH) ** -0.5
EPS = 1e-5
NEG = -1e30


def build_nc(repeat=1):
    nc = bacc.Bacc(None, target_bir_lowering=False, debug=False)

    x_d = nc.dram_tensor("x_in", [N, DIM], DT, kind="ExternalInput")
    wq_d = nc.dram_tensor("wq", [128, KT, H_LOCAL * DH], DT, kind="ExternalInput")
    wk_d = nc.dram_tensor("wk", [128, KT, DH], DT, kind="ExternalInput")
    wv_d = nc.dram_tensor("wv", [128, KT, DH], DT, kind="ExternalInput")
    wo_d = nc.dram_tensor("wo", [128, H_LOCAL, DIM], DT, kind="ExternalInput")
    cos_d = nc.dram_tensor("cosT", [DH, N], DT, kind="ExternalInput")
    sin_d = nc.dram_tensor("sinT", [DH, N], DT, kind="ExternalInput")
    msk_d = nc.dram_tensor("mask", [128, 128], F32, kind="ExternalInput")
    out_d = nc.dram_tensor("out_partial", [N, DIM], DT, kind="ExternalOutput")

    with tile.TileContext(nc) as tc:
        with (
            tc.tile_pool(name="const", bufs=1) as const,
            tc.tile_pool(name="xp", bufs=8) as xp,
            tc.tile_pool(name="xnp", bufs=3) as xnp,
            tc.tile_pool(name="xtp", bufs=2) as xtp,
            tc.tile_pool(name="store", bufs=1) as store,
            tc.tile_pool(name="small", bufs=4) as small,
            tc.tile_pool(name="rope", bufs=4) as ropep,
            tc.tile_pool(name="ep", bufs=3) as ep,
            tc.tile_pool(name="bounce", bufs=2) as bounce,
            tc.tile_pool(name="op", bufs=3) as op,
            tc.tile_pool(name="ps", bufs=1, space="PSUM") as ps,
        ):
            # --- constants ---
            # weights go through the idle gpsimd SWDGE queue so chunk-0 x
            # loads (SP hwdge) and LN ops (ACT) are unobstructed at start.
            # Only k/v weights + rope tables + mask are loaded immediately;
            # wq/wo DMAs are emitted later (when first needed) so they don't
            # steal DMA-engine bandwidth from chunk-0 x tiles.
            wq_sb = const.tile([128, KT, H_LOCAL * DH], DT)
            wk_sb = const.tile([128, KT, DH], DT)
            nc.gpsimd.dma_start(wk_sb[:], wk_d[:])
            wv_sb = const.tile([128, KT, DH], DT)
            nc.gpsimd.dma_start(wv_sb[:], wv_d[:])
            wo_sb = const.tile([128, H_LOCAL, DIM], DT)
            cos_sb = const.tile([DH, N], DT)
            nc.gpsimd.dma_start(cos_sb[:], cos_d[:])
            sin_sb = const.tile([DH, N], DT)
            nc.gpsimd.dma_start(sin_sb[:], sin_d[:])
            msk_sb = const.tile([128, 128], F32)
            nc.gpsimd.dma_start(msk_sb[:], msk_d[:])
            ones_mm = const.tile([128, 128], DT)
            nc.vector.memset(ones_mm, 1.0)

            # --- persistent activations (one batch) ---
            qT_sb = store.tile([DH, H_LOCAL, N], DT, tag="qT")
            kT_sb = store.tile([DH, N], DT, tag="kT")
            v_sb = store.tile([128, TT, DH], DT, tag="v")
            aoT_sb = store.tile([DH, H_LOCAL, N], DT, tag="aoT")

            def rope_evict(dst, src_ps, t0, t1):
                # dst = src*cos + rotate_half(src)*sin_signed, src [128, n] PSUM
                n = t1 - t0
                rot = ropep.tile([DH, CHUNK], DT, tag="rot")
                nc.scalar.copy(rot[0:64, :n], src_ps[64:128, :])
                nc.scalar.copy(rot[64:128, :n], src_ps[0:64, :])
                tmp = ropep.tile([DH, CHUNK], DT, tag="tmp")
                nc.vector.tensor_mul(tmp[:, :n], src_ps[:], cos_sb[:, t0:t1])
                rot2 = ropep.tile([DH, CHUNK], DT, tag="rot2")
                nc.vector.tensor_mul(rot2[:, :n], rot[:, :n], sin_sb[:, t0:t1])
                nc.vector.tensor_add(dst, tmp[:, :n], rot2[:, :n])

            def emit_outproj(ocg, tts=None):
                for tt in tts if tts is not None else range(ocg * 4, (ocg + 1) * 4):
                    for dg in range(4):
                        opp = ps.tile([128, 512], F32, tag="opp", bufs=2)
                        for h in range(H_LOCAL):
                            nc.tensor.matmul(
                                opp[:],
                                aoT_sb[:, h, tt * 128 : (tt + 1) * 128],
                                wo_sb[:, h, dg * 512 : (dg + 1) * 512],
                                start=(h == 0),
                                stop=(h == H_LOCAL - 1),
                            )
                        ot = op.tile([128, 512], DT, tag="ot")
                        if dg % 2 == 0:
                            nc.scalar.copy(ot[:], opp[:])
                        else:
                            nc.vector.tensor_copy(ot[:], opp[:])
                        nc.sync.dma_start(
                            out_d[
                                tt * 128 : (tt + 1) * 128,
                                dg * 512 : (dg + 1) * 512,
                            ],
                            ot[:],
                        )

            for _rep in range(repeat):
                for cg in range(NCH):
                    c0 = cg * CHUNK
                    # ---- LN stats + apply + transpose, pipelined per pair of
                    # 128-token tiles so the first transposes start early ----
                    mr = small.tile([128, 4, 2], F32, tag="mr")
                    xnT = xtp.tile([128, KT, CHUNK], DT, tag="xnT")
                    for half in range(2):
                        xts = []
                        for i in range(2):
                            t = half * 2 + i
                            tok0 = c0 + t * 128
                            x_t = xp.tile([128, DIM], DT, tag="x")
                            xts.append(x_t)
                            nc.sync.dma_start(x_t[:], x_d[tok0 : tok0 + 128, :])
                            stats = small.tile([128, 4, 6], F32, tag="stats")
                            for w in range(4):
                                nc.vector.bn_stats(
                                    out=stats[:, w, :],
                                    in_=x_t[:, w * 512 : (w + 1) * 512],
                                )
                            nc.vector.bn_aggr(out=mr[:, t, :], in_=stats[:])
                        # rstd = rsqrt(var+eps) on DVE only (bit-trick seed +
                        # 2 Newton steps): ScalarE never needs sqrt/ln act
                        # tables — Exp stays resident, zero table reloads.
                        mrh = mr[:, half * 2 : half * 2 + 2, :]
                        v2t = small.tile([128, 2], F32, tag="v2t")
                        nc.vector.tensor_scalar(
                            out=v2t[:], in0=mrh[:, :, 1],
                            scalar1=EPS, scalar2=0.0,
                            op0=mybir.AluOpType.add, op1=mybir.AluOpType.add,
                        )
                        y0 = small.tile([128, 2], F32, tag="y0")
                        nc.vector.tensor_scalar(
                            out=y0[:].bitcast(mybir.dt.int32),
                            in0=v2t[:].bitcast(mybir.dt.int32),
                            scalar1=1, scalar2=-1,
                            op0=mybir.AluOpType.logical_shift_right,
                            op1=mybir.AluOpType.bitwise_xor,
                        )
                        nc.vector.tensor_scalar(
                            out=y0[:].bitcast(mybir.dt.int32),
                            in0=y0[:].bitcast(mybir.dt.int32),
                            scalar1=0x5F3759E0, scalar2=0,
                            op0=mybir.AluOpType.add, op1=mybir.AluOpType.add,
                        )
                        rstd2 = y0
                        for _nr in range(2):
                            a = small.tile([128, 2], F32, tag=f"nr{_nr}")
                            nc.vector.tensor_mul(a[:], rstd2[:], rstd2[:])
                            nc.vector.tensor_mul(a[:], a[:], v2t[:])
                            nc.vector.tensor_scalar(
                                out=a[:], in0=a[:],
                                scalar1=-0.5, scalar2=1.5,
                                op0=mybir.AluOpType.mult, op1=mybir.AluOpType.add,
                            )
                            yn = small.tile([128, 2], F32, tag=f"y{_nr + 1}")
                            nc.vector.tensor_mul(yn[:], a[:], rstd2[:])
                            rstd2 = yn
                        for i in range(2):
                            t = half * 2 + i
                            x_t = xts[i]
                            xn_t = xnp.tile([128, DIM], DT, tag="xn")
                            if t % 2 == 0:
                                nc.vector.tensor_scalar(
                                    out=xn_t[:],
                                    in0=x_t[:],
                                    scalar1=mr[:, t, 0:1],
                                    scalar2=rstd2[:, i : i + 1],
                                    op0=mybir.AluOpType.subtract,
                                    op1=mybir.AluOpType.mult,
                                )
                            else:
                                negmur = small.tile([128, 1], F32, tag="negmur")
                                nc.vector.tensor_scalar(
                                    out=negmur[:],
                                    in0=mr[:, t, 0:1],
                                    scalar1=rstd2[:, i : i + 1],
                                    scalar2=-1.0,
                                    op0=mybir.AluOpType.mult,
                                    op1=mybir.AluOpType.mult,
                                )
                                nc.scalar.activation(
                                    out=xn_t[:],
                                    in_=x_t[:],
                                    func=mybir.ActivationFunctionType.Identity,
                                    bias=negmur[:],
                                    scale=rstd2[:, i : i + 1],
                                )
                            nc.sync.dma_start_transpose(
                                xnT[:, :, t * 128 : (t + 1) * 128], xn_t[:]
                            )
                        if _rep == 0 and cg == 0 and half == 0:
                            # wq arrives while stats/LN half-1 + k/v sweep run
                            nc.gpsimd.dma_start(wq_sb[:], wq_d[:])

                    # ---- projections: 3 sweeps of 2 accumulators ----
                    ktp = ps.tile([DH, CHUNK], F32, tag="acc", bufs=2)
                    vtp = ps.tile([DH, CHUNK], F32, tag="acc", bufs=2)
                    for kt in range(KT):
                        rhs = xnT[:, kt, :]
                        nc.tensor.matmul(
                            ktp[:], wk_sb[:, kt, :], rhs,
                            start=(kt == 0), stop=(kt == KT - 1),
                        )
                        nc.tensor.matmul(
                            vtp[:], wv_sb[:, kt, :], rhs,
                            start=(kt == 0), stop=(kt == KT - 1),
                        )
                    rope_evict(kT_sb[:, c0 : c0 + CHUNK], ktp, c0, c0 + CHUNK)
                    vT_sb = bounce.tile([DH, CHUNK], DT, tag="vT")
                    nc.scalar.copy(vT_sb[:], vtp[:])
                    nc.sync.dma_start_transpose(
                        v_sb[:, cg * 4 : (cg + 1) * 4, :], vT_sb[:]
                    )
                    # out-proj of the PREVIOUS chunk is interleaved at the
                    # three sweep boundaries: its matmuls fill the PE stalls
                    # while the acc banks drain through rope-evict.
                    if cg > 0:
                        emit_outproj(cg - 1, tts=[(cg - 1) * 4, (cg - 1) * 4 + 1])
                    for hp in range(H_LOCAL // 2):
                        qta = ps.tile([DH, CHUNK], F32, tag="acc", bufs=2)
                        qtb = ps.tile([DH, CHUNK], F32, tag="acc", bufs=2)
                        ha, hb = 2 * hp, 2 * hp + 1
                        for kt in range(KT):
                            rhs = xnT[:, kt, :]
                            nc.tensor.matmul(
                                qta[:], wq_sb[:, kt, ha * DH : (ha + 1) * DH], rhs,
                                start=(kt == 0), stop=(kt == KT - 1),
                            )
                            nc.tensor.matmul(
                                qtb[:], wq_sb[:, kt, hb * DH : (hb + 1) * DH], rhs,
                                start=(kt == 0), stop=(kt == KT - 1),
                            )
                        rope_evict(qT_sb[:, ha, c0 : c0 + CHUNK], qta, c0, c0 + CHUNK)
                        rope_evict(qT_sb[:, hb, c0 : c0 + CHUNK], qtb, c0, c0 + CHUNK)
                        if cg > 0:
                            emit_outproj(cg - 1, tts=[(cg - 1) * 4 + 2 + hp])

                    # ---- attention for q-group qg == cg ----
                    nkt = (cg + 1) * (CHUNK // 128)
                    for h in range(H_LOCAL):
                        avT = ps.tile([DH, CHUNK], F32, tag="av", bufs=1)
                        # sums shares the out-proj bank ring (same shape/tag):
                        # attention and out-proj never overlap in PE order, and
                        # the freed bank pays for a 3rd S tile below.
                        sums = ps.tile([128, CHUNK], F32, tag="opp", bufs=2)
                        ets = {}

                        def av_sums(kt, h=h, avT=avT, sums=sums):
                            off = max(0, kt * 128 - c0)
                            et = ets.pop(kt)
                            nc.tensor.matmul(
                                avT[:, off:],
                                v_sb[:, kt, :],
                                et[:, off:],
                                start=(kt == 0),
                                stop=(kt == nkt - 1),
                            )
                            nc.tensor.matmul(
                                sums[:, off:],
                                ones_mm[:],
                                et[:, off:],
                                start=(kt == 0),
                                stop=(kt == nkt - 1),
                            )

                        # S/exp run 2 k-tiles ahead of AV/sums so the PE never
                        # waits on ScalarE's exp (3 S bufs in flight).
                        for kt in range(nkt):
                            off = max(0, kt * 128 - c0)
                            st = ps.tile([128, CHUNK], F32, tag="s", bufs=3)
                            nc.tensor.matmul(
                                st[:, off:],
                                kT_sb[:, kt * 128 : (kt + 1) * 128],
                                qT_sb[:, h, c0 + off : c0 + CHUNK],
                            )
                            if kt * 128 >= c0:  # diagonal block: causal mask
                                nc.vector.tensor_add(
                                    st[:, off : off + 128],
                                    st[:, off : off + 128],
                                    msk_sb[:],
                                )
                            et = ep.tile([128, CHUNK], DT, tag="et", bufs=4)
                            nc.scalar.activation(
                                out=et[:, off:],
                                in_=st[:, off:],
                                func=mybir.ActivationFunctionType.Exp,
                                scale=SCALE,
                            )
                            ets[kt] = et
                            if kt >= 2:
                                av_sums(kt - 2)
                        for kt in range(max(0, nkt - 2), nkt):
                            av_sums(kt)
                        # sums rows are replicated (M=128 ones), so the
                        # reciprocal is already broadcast — no gpsimd hop
                        rbc = bounce.tile([128, CHUNK], F32, tag="rbc")
                        nc.vector.reciprocal_approx_fast(
                            out=rbc[:], in_=sums[:]
                        )
                        nc.vector.tensor_mul(
                            aoT_sb[:, h, c0 : c0 + CHUNK], avT[:], rbc[:]
                        )
                    if _rep == 0 and cg == 0:
                        nc.gpsimd.dma_start(wo_sb[:], wo_d[:])

                # out-proj of the final chunk has no later sweep to hide in
                emit_outproj(NCH - 1)

    nc.compile()
    return nc


def make_in_maps(x, gamma, Wq, Wkv, Wo):
    x = np.asarray(x, dtype=np.float32)
    g = np.asarray(gamma, dtype=np.float32)
    Wq = np.asarray(Wq, dtype=np.float32) * g[:, None]
    Wkv = np.asarray(Wkv, dtype=np.float32) * g[:, None]
    Wo = np.asarray(Wo, dtype=np.float32)

    t = np.arange(N, dtype=np.float64)
    inv = 1.0 / (10000.0 ** (np.arange(0, DH, 2, dtype=np.float64) / DH))  # [64]
    fr = np.outer(inv, t)  # [d, t]
    cosT = np.ascontiguousarray(
        np.concatenate([np.cos(fr), np.cos(fr)], 0).astype(DT_NP)
    )
    sinT = np.ascontiguousarray(
        np.concatenate([-np.sin(fr), np.sin(fr)], 0).astype(DT_NP)
    )
    mask = np.where(
        np.arange(128)[:, None] > np.arange(128)[None, :], NEG, 0.0
    ).astype(np.float32)

    def pt(w):  # [DIM, M] -> [128, KT, M] partition-major
        return np.ascontiguousarray(
            w.reshape(KT, 128, -1).transpose(1, 0, 2).astype(DT_NP)
        )

    Wk = Wkv[:, :DH]
    Wv = Wkv[:, DH:]
    xb = [np.ascontiguousarray(x[b].astype(DT_NP)) for b in range(B)]
    maps = []
    for c in range(N_CORES):
        b = c // (N_CORES // B_GROUPS)
        hg = c % (N_CORES // B_GROUPS)
        wq_c = pt(Wq[:, hg * H_LOCAL * DH : (hg + 1) * H_LOCAL * DH])
        wo_c = np.ascontiguousarray(
            Wo[hg * H_LOCAL * DH : (hg + 1) * H_LOCAL * DH]
            .reshape(H_LOCAL, DH, DIM)
            .transpose(1, 0, 2)
            .astype(DT_NP)
        )
        maps.append(
            {
                "x_in": xb[b],
                "wq": wq_c,
                "wk": pt(Wk),
                "wv": pt(Wv),
                "wo": wo_c,
                "cosT": cosT,
                "sinT": sinT,
                "mask": mask,
            }
        )
    return maps


_NC_CACHE = {}


def get_nc(repeat=1):
    key = repeat
    if key not in _NC_CACHE:
        _NC_CACHE[key] = build_nc(repeat)
    return _NC_CACHE[key]


def kernel(x, gamma, Wq, Wkv, Wo, _trace=False, _repeat=1):
    from concourse import bass_utils

    nc = get_nc(_repeat)
    in_maps = make_in_maps(x, gamma, Wq, Wkv, Wo)
    res = bass_utils.run_bass_kernel_spmd(
        nc, in_maps, core_ids=list(range(N_CORES)), trace=_trace
    )
    out = np.zeros((B, N, DIM), dtype=np.float32)
    per_b = N_CORES // B_GROUPS
    for c, r in enumerate(res.results):
        out[c // per_b] += np.asarray(r["out_partial"], dtype=np.float32)
    if _trace:
        kernel.last_results = res
    return out


# revision 20
# speedup vs baseline: 1.7998x; 1.0181x over previous
"""MQA attention (LN + QKV proj + RoPE + causal attn + out-proj) on 8 trn2 cores.

Sharding: 2D batch x head-group. Core c handles batch c//4 and heads
4*(c%4) .. 4*(c%4)+4 (Wq cols + Wo rows). K/V (single MQA head) is computed
redundantly per core for its batch. Out-proj produces per-core partials which
the host reduces (4 partials per batch).

Per-core dataflow (bf16 matmuls, f32 accumulation):
  LN(x) natural layout -> DMA-xbar transpose (SP-dispatched, runs on DMA
  engines) to xn^T -> q^T (4 heads) / k^T / v^T projections -> RoPE on
  q^T,k^T -> per 512-token q-group: S^T = k @ q^T causal-blocked, exp on
  ScalarE (scale folded; |S*scale| small enough to skip max subtraction),
  AV accumulation + row sums via ones-matmul -> normalize with
  reciprocal_approx_fast + partition_broadcast -> out-proj partial.

ScalarE uses only {Ln, Exp, Identity, Copy} which live in ONE activation
table set (natural_log_exp_and_others), so there are no table reloads even
with phases interleaved; rstd = exp(-0.5*ln(var+eps)).
"""

import sys

if "/opt/trn_rl_repo" not in sys.path:
    sys.path.insert(0, "/opt/trn_rl_repo")

import ml_dtypes
import numpy as np

import concourse.bass as bass
import concourse.tile as tile
from concourse import bacc, mybir

F32 = mybir.dt.float32
DT = mybir.dt.bfloat16  # matmul operand storage dtype
DT_NP = ml_dtypes.bfloat16

B, N, DIM, DH, HEADS = 2, 2048, 2048, 128, 16
H_LOCAL = 4  # heads per core
B_GROUPS = 2  # batch split
N_CORES = 8
KT = DIM // 128  # k-tiles over the model dim
TT = N // 128  # token tiles (one batch)
CHUNK = 512  # token chunk for projection phase + q-group width
NCH = N // CHUNK  # chunks per batch
SCALE = float(DH) ** -0.5
EPS = 1e-5
NEG = -1e30


def build_nc(repeat=1):
    nc = bacc.Bacc(None, target_bir_lowering=False, debug=False)

    x_d = nc.dram_tensor("x_in", [N, DIM], DT, kind="ExternalInput")
    wq_d = nc.dram_tensor("wq", [128, KT, H_LOCAL * DH], DT, kind="ExternalInput")
    wk_d = nc.dram_tensor("wk", [128, KT, DH], DT, kind="ExternalInput")
    wv_d = nc.dram_tensor("wv", [128, KT, DH], DT, kind="ExternalInput")
    wo_d = nc.dram_tensor("wo", [128, H_LOCAL, DIM], DT, kind="ExternalInput")
    cos_d = nc.dram_tensor("cosT", [DH, N], DT, kind="ExternalInput")
    sin_d = nc.dram_tensor("sinT", [DH, N], DT, kind="ExternalInput")
    msk_d = nc.dram_tensor("mask", [128, 128], F32, kind="ExternalInput")
    out_d = nc.dram_tensor("out_partial", [N, DIM], DT, kind="ExternalOutput")

    with tile.TileContext(nc) as tc:
        with (
            tc.tile_pool(name="const", bufs=1) as const,
            tc.tile_pool(name="xp", bufs=4) as xp,
            tc.tile_pool(name="xnp", bufs=3) as xnp,
            tc.tile_pool(name="xtp", bufs=2) as xtp,
            tc.tile_pool(name="store", bufs=1) as store,
            tc.tile_pool(name="small", bufs=4) as small,
            tc.tile_pool(name="rope", bufs=4) as ropep,
            tc.tile_pool(name="ep", bufs=3) as ep,
            tc.tile_pool(name="bounce", bufs=2) as bounce,
            tc.tile_pool(name="op", bufs=3) as op,
            tc.tile_pool(name="ps", bufs=1, space="PSUM") as ps,
        ):
            # --- constants ---
            # weights go through the idle gpsimd SWDGE queue so chunk-0 x
            # loads (SP hwdge) and LN ops (ACT) are unobstructed at start.
            # Only k/v weights + rope tables + mask are loaded immediately;
            # wq/wo DMAs are emitted later (when first needed) so they don't
            # steal DMA-engine bandwidth from chunk-0 x tiles.
            wq_sb = const.tile([128, KT, H_LOCAL * DH], DT)
            wk_sb = const.tile([128, KT, DH], DT)
            nc.gpsimd.dma_start(wk_sb[:], wk_d[:])
            wv_sb = const.tile([128, KT, DH], DT)
            nc.gpsimd.dma_start(wv_sb[:], wv_d[:])
            wo_sb = const.tile([128, H_LOCAL, DIM], DT)
            cos_sb = const.tile([DH, N], DT)
            nc.gpsimd.dma_start(cos_sb[:], cos_d[:])
            sin_sb = const.tile([DH, N], DT)
            nc.gpsimd.dma_start(sin_sb[:], sin_d[:])
            msk_sb = const.tile([128, 128], F32)
            nc.gpsimd.dma_start(msk_sb[:], msk_d[:])
            ones_mm = const.tile([128, 128], DT)
            nc.vector.memset(ones_mm, 1.0)

            # --- persistent activations (one batch) ---
            qT_sb = store.tile([DH, H_LOCAL, N], DT, tag="qT")
            kT_sb = store.tile([DH, N], DT, tag="kT")
            v_sb = store.tile([128, TT, DH], DT, tag="v")
            aoT_sb = store.tile([DH, H_LOCAL, N], DT, tag="aoT")

            def rope_evict(dst, src_ps, t0, t1):
                # dst = src*cos + rotate_half(src)*sin_signed, src [128, n] PSUM
                n = t1 - t0
                rot = ropep.tile([DH, CHUNK], DT, tag="rot")
                nc.scalar.copy(rot[0:64, :n], src_ps[64:128, :])
                nc.scalar.copy(rot[64:128, :n], src_ps[0:64, :])
                tmp = ropep.tile([DH, CHUNK], DT, tag="tmp")
                nc.vector.tensor_mul(tmp[:, :n], src_ps[:], cos_sb[:, t0:t1])
                rot2 = ropep.tile([DH, CHUNK], DT, tag="rot2")
                nc.vector.tensor_mul(rot2[:, :n], rot[:, :n], sin_sb[:, t0:t1])
                nc.vector.tensor_add(dst, tmp[:, :n], rot2[:, :n])

            def emit_outproj(ocg, tts=None):
                for tt in tts if tts is not None else range(ocg * 4, (ocg + 1) * 4):
                    for dg in range(4):
                        opp = ps.tile([128, 512], F32, tag="opp", bufs=2)
                        for h in range(H_LOCAL):
                            nc.tensor.matmul(
                                opp[:],
                                aoT_sb[:, h, tt * 128 : (tt + 1) * 128],
                                wo_sb[:, h, dg * 512 : (dg + 1) * 512],
                                start=(h == 0),
                                stop=(h == H_LOCAL - 1),
                            )
                        ot = op.tile([128, 512], DT, tag="ot")
                        if dg % 2 == 0:
                            nc.scalar.copy(ot[:], opp[:])
                        else:
                            nc.vector.tensor_copy(ot[:], opp[:])
                        nc.sync.dma_start(
                            out_d[
                                tt * 128 : (tt + 1) * 128,
                                dg * 512 : (dg + 1) * 512,
                            ],
                            ot[:],
                        )

            for _rep in range(repeat):
                for cg in range(NCH):
                    c0 = cg * CHUNK
                    # ---- LN stats + apply + transpose, pipelined per pair of
                    # 128-token tiles so the first transposes start early ----
                    mr = small.tile([128, 4, 2], F32, tag="mr")
                    xnT = xtp.tile([128, KT, CHUNK], DT, tag="xnT")
                    for half in range(2):
                        xts = []
                        for i in range(2):
                            t = half * 2 + i
                            tok0 = c0 + t * 128
                            x_t = xp.tile([128, DIM], DT, tag="x")
                            xts.append(x_t)
                            nc.sync.dma_start(x_t[:], x_d[tok0 : tok0 + 128, :])
                            stats = small.tile([128, 4, 6], F32, tag="stats")
                            for w in range(4):
                                nc.vector.bn_stats(
                                    out=stats[:, w, :],
                                    in_=x_t[:, w * 512 : (w + 1) * 512],
                                )
                            nc.vector.bn_aggr(out=mr[:, t, :], in_=stats[:])
                        # rstd = rsqrt(var+eps) on DVE only (bit-trick seed +
                        # 2 Newton steps): ScalarE never needs sqrt/ln act
                        # tables — Exp stays resident, zero table reloads.
                        mrh = mr[:, half * 2 : half * 2 + 2, :]
                        v2t = small.tile([128, 2], F32, tag="v2t")
                        nc.vector.tensor_scalar(
                            out=v2t[:], in0=mrh[:, :, 1],
                            scalar1=EPS, scalar2=0.0,
                            op0=mybir.AluOpType.add, op1=mybir.AluOpType.add,
                        )
                        y0 = small.tile([128, 2], F32, tag="y0")
                        nc.vector.tensor_scalar(
                            out=y0[:].bitcast(mybir.dt.int32),
                            in0=v2t[:].bitcast(mybir.dt.int32),
                            scalar1=1, scalar2=-1,
                            op0=mybir.AluOpType.logical_shift_right,
                            op1=mybir.AluOpType.bitwise_xor,
                        )
                        nc.vector.tensor_scalar(
                            out=y0[:].bitcast(mybir.dt.int32),
                            in0=y0[:].bitcast(mybir.dt.int32),
                            scalar1=0x5F3759E0, scalar2=0,
                            op0=mybir.AluOpType.add, op1=mybir.AluOpType.add,
                        )
                        rstd2 = y0
                        for _nr in range(2):
                            a = small.tile([128, 2], F32, tag=f"nr{_nr}")
                            nc.vector.tensor_mul(a[:], rstd2[:], rstd2[:])
                            nc.vector.tensor_mul(a[:], a[:], v2t[:])
                            nc.vector.tensor_scalar(
                                out=a[:], in0=a[:],
                                scalar1=-0.5, scalar2=1.5,
                                op0=mybir.AluOpType.mult, op1=mybir.AluOpType.add,
                            )
                            yn = small.tile([128, 2], F32, tag=f"y{_nr + 1}")
                            nc.vector.tensor_mul(yn[:], a[:], rstd2[:])
                            rstd2 = yn
                        for i in range(2):
                            t = half * 2 + i
                            x_t = xts[i]
                            xn_t = xnp.tile([128, DIM], DT, tag="xn")
                            if t % 2 == 0:
                                nc.vector.tensor_scalar(
                                    out=xn_t[:],
                                    in0=x_t[:],
                                    scalar1=mr[:, t, 0:1],
                                    scalar2=rstd2[:, i : i + 1],
                                    op0=mybir.AluOpType.subtract,
                                    op1=mybir.AluOpType.mult,
                                )
                            else:
                                negmur = small.tile([128, 1], F32, tag="negmur")
                                nc.vector.tensor_scalar(
                                    out=negmur[:],
                                    in0=mr[:, t, 0:1],
                                    scalar1=rstd2[:, i : i + 1],
                                    scalar2=-1.0,
                                    op0=mybir.AluOpType.mult,
                                    op1=mybir.AluOpType.mult,
                                )
                                nc.scalar.activation(
                                    out=xn_t[:],
                                    in_=x_t[:],
                                    func=mybir.ActivationFunctionType.Identity,
                                    bias=negmur[:],
                                    scale=rstd2[:, i : i + 1],
                                )
                            nc.sync.dma_start_transpose(
                                xnT[:, :, t * 128 : (t + 1) * 128], xn_t[:]
                            )
                        if _rep == 0 and cg == 0 and half == 0:
                            # wq arrives while stats/LN half-1 + k/v sweep run
                            nc.gpsimd.dma_start(wq_sb[:], wq_d[:])

                    # ---- projections: 3 sweeps of 2 accumulators ----
                    ktp = ps.tile([DH, CHUNK], F32, tag="acc", bufs=2)
                    vtp = ps.tile([DH, CHUNK], F32, tag="acc", bufs=2)
                    # sweeps run per half-chunk so the PE starts as soon as
                    # the first two DMA transposes land (a later start=True
                    # only clears has_written bits; finished data persists)
                    for hb in range(2):
                        cl, ch = hb * 256, hb * 256 + 256
                        for kt in range(KT):
                            rhs = xnT[:, kt, cl:ch]
                            nc.tensor.matmul(
                                ktp[:, cl:ch], wk_sb[:, kt, :], rhs,
                                start=(kt == 0), stop=(kt == KT - 1),
                            )
                            nc.tensor.matmul(
                                vtp[:, cl:ch], wv_sb[:, kt, :], rhs,
                                start=(kt == 0), stop=(kt == KT - 1),
                            )
                    rope_evict(kT_sb[:, c0 : c0 + CHUNK], ktp, c0, c0 + CHUNK)
                    vT_sb = bounce.tile([DH, CHUNK], DT, tag="vT")
                    nc.scalar.copy(vT_sb[:], vtp[:])
                    nc.sync.dma_start_transpose(
                        v_sb[:, cg * 4 : (cg + 1) * 4, :], vT_sb[:]
                    )
                    # out-proj of the PREVIOUS chunk is interleaved at the
                    # three sweep boundaries: its matmuls fill the PE stalls
                    # while the acc banks drain through rope-evict.
                    if cg > 0:
                        emit_outproj(cg - 1, tts=[(cg - 1) * 4, (cg - 1) * 4 + 1])
                    for hp in range(H_LOCAL // 2):
                        qta = ps.tile([DH, CHUNK], F32, tag="acc", bufs=2)
                        qtb = ps.tile([DH, CHUNK], F32, tag="acc", bufs=2)
                        ha, hq = 2 * hp, 2 * hp + 1
                        for hb in range(2):
                            cl, ch = hb * 256, hb * 256 + 256
                            for kt in range(KT):
                                rhs = xnT[:, kt, cl:ch]
                                nc.tensor.matmul(
                                    qta[:, cl:ch],
                                    wq_sb[:, kt, ha * DH : (ha + 1) * DH], rhs,
                                    start=(kt == 0), stop=(kt == KT - 1),
                                )
                                nc.tensor.matmul(
                                    qtb[:, cl:ch],
                                    wq_sb[:, kt, hq * DH : (hq + 1) * DH], rhs,
                                    start=(kt == 0), stop=(kt == KT - 1),
                                )
                        rope_evict(qT_sb[:, ha, c0 : c0 + CHUNK], qta, c0, c0 + CHUNK)
                        rope_evict(qT_sb[:, hq, c0 : c0 + CHUNK], qtb, c0, c0 + CHUNK)
                        if cg > 0:
                            emit_outproj(cg - 1, tts=[(cg - 1) * 4 + 2 + hp])

                    # ---- attention for q-group qg == cg ----
                    nkt = (cg + 1) * (CHUNK // 128)
                    for h in range(H_LOCAL):
                        avT = ps.tile([DH, CHUNK], F32, tag="av", bufs=1)
                        # sums shares the out-proj bank ring (same shape/tag):
                        # attention and out-proj never overlap in PE order, and
                        # the freed bank pays for a 3rd S tile below.
                        sums = ps.tile([128, CHUNK], F32, tag="opp", bufs=2)
                        ets = {}

                        def av_sums(kt, h=h, avT=avT, sums=sums):
                            off = max(0, kt * 128 - c0)
                            et = ets.pop(kt)
                            nc.tensor.matmul(
                                avT[:, off:],
                                v_sb[:, kt, :],
                                et[:, off:],
                                start=(kt == 0),
                                stop=(kt == nkt - 1),
                            )
                            nc.tensor.matmul(
                                sums[:, off:],
                                ones_mm[:],
                                et[:, off:],
                                start=(kt == 0),
                                stop=(kt == nkt - 1),
                            )

                        # S/exp run 2 k-tiles ahead of AV/sums so the PE never
                        # waits on ScalarE's exp (3 S bufs in flight).
                        for kt in range(nkt):
                            off = max(0, kt * 128 - c0)
                            st = ps.tile([128, CHUNK], F32, tag="s", bufs=3)
                            nc.tensor.matmul(
                                st[:, off:],
                                kT_sb[:, kt * 128 : (kt + 1) * 128],
                                qT_sb[:, h, c0 + off : c0 + CHUNK],
                            )
                            if kt * 128 >= c0:  # diagonal block: causal mask
                                nc.vector.tensor_add(
                                    st[:, off : off + 128],
                                    st[:, off : off + 128],
                                    msk_sb[:],
                                )
                            et = ep.tile([128, CHUNK], DT, tag="et", bufs=4)
                            nc.scalar.activation(
                                out=et[:, off:],
                                in_=st[:, off:],
                                func=mybir.ActivationFunctionType.Exp,
                                scale=SCALE,
                            )
                            ets[kt] = et
                            if kt >= 2:
                                av_sums(kt - 2)
                        for kt in range(max(0, nkt - 2), nkt):
                            av_sums(kt)
                        # sums rows are replicated (M=128 ones), so the
                        # reciprocal is already broadcast — no gpsimd hop
                        rbc = bounce.tile([128, CHUNK], F32, tag="rbc")
                        nc.vector.reciprocal_approx_fast(
                            out=rbc[:], in_=sums[:]
                        )
                        nc.vector.tensor_mul(
                            aoT_sb[:, h, c0 : c0 + CHUNK], avT[:], rbc[:]
                        )
                    if _rep == 0 and cg == 0:
                        nc.gpsimd.dma_start(wo_sb[:], wo_d[:])

                # out-proj of the final chunk has no later sweep to hide in
                emit_outproj(NCH - 1)

    nc.compile()
    return nc


def make_in_maps(x, gamma, Wq, Wkv, Wo):
    x = np.asarray(x, dtype=np.float32)
    g = np.asarray(gamma, dtype=np.float32)
    Wq = np.asarray(Wq, dtype=np.float32) * g[:, None]
    Wkv = np.asarray(Wkv, dtype=np.float32) * g[:, None]
    Wo = np.asarray(Wo, dtype=np.float32)

    t = np.arange(N, dtype=np.float64)
    inv = 1.0 / (10000.0 ** (np.arange(0, DH, 2, dtype=np.float64) / DH))  # [64]
    fr = np.outer(inv, t)  # [d, t]
    cosT = np.ascontiguousarray(
        np.concatenate([np.cos(fr), np.cos(fr)], 0).astype(DT_NP)
    )
    sinT = np.ascontiguousarray(
        np.concatenate([-np.sin(fr), np.sin(fr)], 0).astype(DT_NP)
    )
    mask = np.where(
        np.arange(128)[:, None] > np.arange(128)[None, :], NEG, 0.0
    ).astype(np.float32)

    def pt(w):  # [DIM, M] -> [128, KT, M] partition-major
        return np.ascontiguousarray(
            w.reshape(KT, 128, -1).transpose(1, 0, 2).astype(DT_NP)
        )

    Wk = Wkv[:, :DH]
    Wv = Wkv[:, DH:]
    xb = [np.ascontiguousarray(x[b].astype(DT_NP)) for b in range(B)]
    maps = []
    for c in range(N_CORES):
        b = c // (N_CORES // B_GROUPS)
        hg = c % (N_CORES // B_GROUPS)
        wq_c = pt(Wq[:, hg * H_LOCAL * DH : (hg + 1) * H_LOCAL * DH])
        wo_c = np.ascontiguousarray(
            Wo[hg * H_LOCAL * DH : (hg + 1) * H_LOCAL * DH]
            .reshape(H_LOCAL, DH, DIM)
            .transpose(1, 0, 2)
            .astype(DT_NP)
        )
        maps.append(
            {
                "x_in": xb[b],
                "wq": wq_c,
                "wk": pt(Wk),
                "wv": pt(Wv),
                "wo": wo_c,
                "cosT": cosT,
                "sinT": sinT,
                "mask": mask,
            }
        )
    return maps


_NC_CACHE = {}


def get_nc(repeat=1):
    key = repeat
    if key not in _NC_CACHE:
        _NC_CACHE[key] = build_nc(repeat)
    return _NC_CACHE[key]


def kernel(x, gamma, Wq, Wkv, Wo, _trace=False, _repeat=1):
    from concourse import bass_utils

    nc = get_nc(_repeat)
    in_maps = make_in_maps(x, gamma, Wq, Wkv, Wo)
    res = bass_utils.run_bass_kernel_spmd(
        nc, in_maps, core_ids=list(range(N_CORES)), trace=_trace
    )
    out = np.zeros((B, N, DIM), dtype=np.float32)
    per_b = N_CORES // B_GROUPS
    for c, r in enumerate(res.results):
        out[c // per_b] += np.asarray(r["out_partial"], dtype=np.float32)
    if _trace:
        kernel.last_results = res
    return out
